# revision 32
# baseline (speedup 1.0000x reference)
"""CrossViT fused block on 8 TRN2 NeuronCores.

Sharding: 2 branches (vis-output / ir-output) x 4-way token split -> 8 cores,
no collectives. Each core computes 49 output tokens of one branch end-to-end:
LN1, cross-attention (its queries vs all 196 keys/values of the other
modality), projection, residual, LN2, FFN, residual. Activations are kept
feature-major (features on SBUF partitions) so every linear layer is
matmul(lhsT=W_natural, rhs=xT). Matmul operands bf16 (fp32 PSUM).

vs. the first version: GpSimd is not used at all (its microcode library
swaps cost ~5us each); softmax normalization is a DVE tensor_scalar divide
by the per-partition exp-sum; LayerNorm mean/rstd are broadcast across
partitions with tiny PE outer-product matmuls against a ones row; ACT
tables (Exp/Sqrt/Gelu) are preloaded with dummy activations during the
input DMA; inputs arrive as 6 separately-DMA'd blobs ordered by first use;
q is computed densely then scattered lane-aligned into block-diag pair
tiles; k and attnV matmuls are pair-merged; LN2's mean is derived from
colsum(Wp) @ oT before the residual even lands.
"""
import sys
if '/opt/trn_rl_repo' not in sys.path:
    sys.path.insert(0, '/opt/trn_rl_repo')

import numpy as np
import ml_dtypes

BF = ml_dtypes.bfloat16
N, EMB, H, DH, HID = 196, 256, 8, 32, 1024
T = 49            # tokens per core
EPS, SCALE = 1e-5, 16.0
P = 128
NCORES = 8
TOKC = ((0, 128), (128, 68))   # token chunks of the 196 keys/values

# blobA f32 cols: [0:98] xq | [98:130] aux | [130:179] id98(bf16) |
#                 [179:435] wq(bf16) | [435:436] wpcs(bf16) |
#                 [436:692] wk(bf16) | [692:888] xkv(bf16)
CA = 888
# blobB: [0:256] wv(bf16) | [256:384] bv replicated (bf16) | [384:640] wp(bf16)
CB = 640
CC, CD = 1024, 1024   # w1, w2 (bf16)

_CACHE = {}


# ---------------------------------------------------------------- bass build
def build_bass():
    import concourse.bacc as bacc
    import concourse.mybir as mybir
    import concourse.tile as tile

    f32 = mybir.dt.float32
    bf16 = mybir.dt.bfloat16
    AF = mybir.ActivationFunctionType
    OP = mybir.AluOpType

    nc = bacc.Bacc("TRN2", target_bir_lowering=False)

    bA_d = nc.dram_tensor("blobA", [P, CA], f32, kind="ExternalInput")
    bB_d = nc.dram_tensor("blobB", [P, CB], f32, kind="ExternalInput")
    bC_d = nc.dram_tensor("blobC", [P, CC], f32, kind="ExternalInput")
    bD_d = nc.dram_tensor("blobD", [P, CD], f32, kind="ExternalInput")
    out_d = nc.dram_tensor("out", [EMB, T], f32, kind="ExternalOutput")

    with tile.TileContext(nc) as tc:
        with (
            tc.tile_pool(name="const", bufs=1) as cpool,
            tc.tile_pool(name="act", bufs=1) as apool,
            tc.tile_pool(name="attp", bufs=2) as attpool,
            tc.tile_pool(name="ps_mm", bufs=2, space="PSUM") as ps_mm,
            tc.tile_pool(name="ps_s", bufs=2, space="PSUM") as ps_s,
            tc.tile_pool(name="ps_t", bufs=2, space="PSUM") as ps_t,
            tc.tile_pool(name="ps_big", bufs=1, space="PSUM") as ps_big,
            tc.tile_pool(name="ps_ln", bufs=1, space="PSUM") as ps_ln,
        ):
            # ---- constants / scratch (no input deps; runs during DMA wait)
            ones_row = cpool.tile([1, P], bf16, tag="ones_row")
            nc.vector.memset(ones_row[:], 1.0)
            ones_col = cpool.tile([P, 1], bf16, tag="ones_col")
            nc.vector.memset(ones_col[:], 1.0)
            epst = cpool.tile([1, 1], f32, tag="epst")
            nc.vector.memset(epst[:], EPS)
            # block-diag q tiles: bdt[0] holds pairs 0,1 / bdt[1] pairs 2,3
            bdt = []
            for i in range(2):
                bd = apool.tile([P, 2 * T], bf16, tag=f"bd{i}")
                nc.vector.memset(bd[:], 0.0)
                bdt.append(bd)

            # ---- input DMAs, ordered by first use
            bA = cpool.tile([P, CA], f32, tag="bA")
            nc.sync.dma_start(bA[:], bA_d[:, :])
            bB = cpool.tile([P, CB], f32, tag="bB")
            nc.sync.dma_start(bB[:], bB_d[:, :])
            bC = cpool.tile([P, CC], f32, tag="bC")
            nc.sync.dma_start(bC[:], bC_d[:, :])
            bD = cpool.tile([P, CD], f32, tag="bD")
            nc.sync.dma_start(bD[:], bD_d[:, :])

            # ---- views
            xq3 = bA[:, 0:98].rearrange("p (c t) -> p c t", c=2)
            aux = bA[:, 98:130]
            id98 = bA[:, 130:179].bitcast(bf16)[0:98, 0:98]
            wq = bA[:, 179:435].bitcast(bf16).rearrange("p (c m) -> p c m", c=2)
            wpcs = bA[:, 435:436].bitcast(bf16)  # [128, 2]
            bq_c = aux[:, 0:2]
            bk_c = aux[:, 2:4]
            bp_c = aux[:, 4:6]
            b2_c = aux[:, 6:8]
            b1_c = aux[:, 8:16]
            sumbp = aux[0:1, 16:17]
            ln1b_c = aux[:, 17:19]
            ln2b_c = aux[:, 19:21]
            ln1w_c = aux[:, 21:23]
            ln2w_c = aux[:, 23:25]
            wk = bA[:, 436:692].bitcast(bf16).rearrange("p (c m) -> p c m", c=2)
            xkv = bA[:, 692:CA].bitcast(bf16).rearrange("p (c t) -> p c t", c=2)
            wv = bB[:, 0:256].bitcast(bf16).rearrange("p (c m) -> p c m", c=2)
            bvv = bB[:, 256:384].bitcast(bf16)
            wp = bB[:, 384:CB].bitcast(bf16).rearrange("p (c m) -> p c m", c=2)
            w1 = bC[:, 0:CC].bitcast(bf16).rearrange("p (c m) -> p c m", c=2)
            w2 = bD[:, 0:CD].bitcast(bf16).rearrange("p (c m) -> p c m", c=8)

            xq_bf = apool.tile([P, 2, T], bf16, tag="xq_bf")
            nc.vector.tensor_copy(xq_bf[:], xq3)

            # packed LN PSUM bank: col views
            ln = ps_ln.tile([P, 512], f32, tag="ln")
            lnp1 = ln[0:1, 0:98].rearrange("p (a t) -> p a t", a=2)
            msum = ln[0:1, 98:147]
            ssq2 = ln[0:1, 147:196]
            Mb1 = ln[:, 196:245]
            Mb2 = ln[:, 245:294]
            R1 = ln[:, 294:343]
            R2 = ln[:, 343:392]

            # ---------------- q dense (2 matmuls/chunk), lane-aligned scatter
            # into block-diag pair tiles; bias adds split DVE/ACT
            # (ACT Identity/Copy/Square live in every table -> no reload)
            for mc in range(2):
                qp = ps_mm.tile([P, 512], f32, tag="mm")
                for kc in range(2):
                    nc.tensor.matmul(qp[:, 0:T], wq[:, kc, mc * P:(mc + 1) * P],
                                     xq_bf[:, kc], start=(kc == 0), stop=(kc == 1))
                for hh in range(4):
                    j = hh % 2           # position within pair
                    r0 = hh * DH
                    dst = bdt[mc][r0:r0 + DH, j * T:(j + 1) * T]
                    src = qp[r0:r0 + DH, 0:T]
                    bias = bq_c[r0:r0 + DH, mc:mc + 1]
                    if hh % 2 == 0:
                        nc.vector.tensor_scalar(dst, src, bias, None, op0=OP.add)
                    else:
                        nc.scalar.add(dst, src, bias)

            # ---------------- k pair-merged: [128, 196] per half (2 pairs)
            # pk tiles borrow the big/attT PSUM slots (their later users start
            # well after the k bias reads) so k doesn't WAR-wait on q's slots
            kt = []
            for i in range(2):
                pool = ps_big if i == 0 else ps_t
                pk = pool.tile([P, 512], f32, tag="big" if i == 0 else "attT")
                for kc in range(2):
                    nc.tensor.matmul(pk[:, 0:N], wk[:, kc, i * P:(i + 1) * P],
                                     xkv[:, kc], start=(kc == 0), stop=(kc == 1))
                k = apool.tile([P, N], bf16, tag=f"k{i}")
                nc.scalar.add(k[:], pk[:, 0:N], bk_c[:, i:i + 1])
                kt.append(k)

            # ---------------- v token-major
            v_bf = []
            for tcx, (t0, tsz) in enumerate(TOKC):
                pv = ps_mm.tile([P, 512], f32, tag="mm")
                for kc in range(2):
                    nc.tensor.matmul(pv[:tsz, 0:EMB], xkv[:, kc, t0:t0 + tsz],
                                     wv[:, kc], start=(kc == 0), stop=(kc == 1))
                vt = apool.tile([P, EMB], bf16, tag=f"v{tcx}")
                nc.vector.tensor_tensor(vt[:tsz], pv[:tsz, 0:EMB], bvv[0:tsz],
                                        op=OP.add)
                v_bf.append(vt)

            # ---------------- LN1 stats (off critical path)
            xsq1 = apool.tile([P, 2, 2, T], bf16, tag="xsq1")
            nc.vector.tensor_copy(xsq1[:, :, 0], xq_bf[:])
            nc.vector.tensor_tensor(xsq1[:, :, 1], xq_bf[:], xq_bf[:], op=OP.mult)
            for kc in range(2):
                nc.tensor.matmul(lnp1.rearrange("p a t -> p (a t)"), ones_col[:],
                                 xsq1[:, kc].rearrange("p a t -> p (a t)"),
                                 start=(kc == 0), stop=(kc == 1))

            # ---------------- scores + softmax (DVE reciprocal + scale)
            ssum = apool.tile([2 * T, 4], f32, tag="ssum")
            rsum = apool.tile([2 * T, 4], f32, tag="rsum")
            att_tiles = []
            for pr in range(4):
                prow = (pr % 2) * 64
                pss = ps_s.tile([2 * T, 512], f32, tag="scores")
                nc.tensor.matmul(pss[:, 0:N], bdt[pr // 2][prow:prow + 64, :],
                                 kt[pr // 2][prow:prow + 64, :],
                                 start=True, stop=True)
                atf = attpool.tile([2 * T, N], bf16, tag="attf")
                nc.scalar.activation(atf[:], pss[:, 0:N], AF.Exp, scale=1.0 / SCALE,
                                     accum_out=ssum[:, pr:pr + 1])
                nc.vector.reciprocal(rsum[:, pr:pr + 1], ssum[:, pr:pr + 1])
                att = attpool.tile([2 * T, N], bf16, tag="attn")
                nc.vector.tensor_scalar(att[:], atf[:], rsum[:, pr:pr + 1],
                                        None, op0=OP.mult)
                att_tiles.append(att)

            # ---------------- attT via PE transpose; pair-merged attnV
            oT_bf = apool.tile([P, 2, T], bf16, tag="oT_bf")
            sc = ps_big.tile([P, 512], f32, tag="big")
            for pr in range(4):
                prow = (pr % 2) * 64
                c0 = (pr // 2) * 2 * T
                ats = []
                for tcx, (t0, tsz) in enumerate(TOKC):
                    pt = ps_t.tile([P, 1024], bf16, tag="attT")
                    nc.tensor.transpose(pt[:tsz, 0:2 * T],
                                        att_tiles[pr][:, t0:t0 + tsz], id98[:])
                    at = attpool.tile([P, 2 * T], bf16, tag="atT")
                    nc.vector.tensor_copy(at[:tsz], pt[:tsz, 0:2 * T])
                    ats.append(at)
                for tcx, (t0, tsz) in enumerate(TOKC):
                    nc.tensor.matmul(sc[prow:prow + 64, c0:c0 + 2 * T],
                                     v_bf[tcx][:tsz, pr * 64:(pr + 1) * 64],
                                     ats[tcx][:tsz], start=(tcx == 0),
                                     stop=(tcx == 1), tile_position=(0, prow))
                nc.scalar.copy(oT_bf[prow:prow + DH, pr // 2],
                               sc[prow:prow + DH, c0:c0 + T])
                nc.scalar.copy(oT_bf[prow + DH:prow + 64, pr // 2],
                               sc[prow + DH:prow + 64, c0 + T:c0 + 2 * T])

            # ---------------- LN1 finish: mean/rstd + outer-product broadcast
            m1 = apool.tile([1, T], f32, tag="m1")
            nc.vector.tensor_scalar(m1[:], lnp1[0:1, 0], 1.0 / EMB, None,
                                    op0=OP.mult)
            m2x1 = apool.tile([1, T], f32, tag="m2x1")
            nc.vector.tensor_tensor(m2x1[:], m1[:], m1[:], op=OP.mult)
            d1 = apool.tile([1, T], f32, tag="d1")
            nc.vector.scalar_tensor_tensor(d1[:], lnp1[0:1, 1], 1.0 / EMB,
                                           m2x1[:], op0=OP.mult, op1=OP.subtract)
            # gate: reads rsum[:,3] so the Sqrt lands after all 4 Exps on the
            # ACT queue (the ACT engine holds one table; interleaving Sqrt
            # between Exps costs a 1.3us table reload each way)
            d1g = apool.tile([1, T], f32, tag="d1g")
            nc.vector.tensor_scalar(d1g[:], d1[:], rsum[0:1, 3:4], None,
                                    op0=OP.bypass)
            vstd1 = apool.tile([1, T], f32, tag="vstd1")
            nc.scalar.activation(vstd1[:], d1g[:], AF.Sqrt, bias=epst[0:1, 0:1])
            r1 = apool.tile([1, T], f32, tag="r1")
            nc.vector.reciprocal_approx_fast(r1[:], vstd1[:])
            mr1_bf = apool.tile([1, 2, T], bf16, tag="mr1_bf")
            nc.vector.tensor_copy(mr1_bf[0:1, 0], m1[:])
            nc.vector.tensor_copy(mr1_bf[0:1, 1], r1[:])
            nc.tensor.matmul(Mb1, ones_row[:], mr1_bf[0:1, 0], start=True,
                             stop=True)
            nc.tensor.matmul(R1, ones_row[:], mr1_bf[0:1, 1], start=True,
                             stop=True)
            t1a = apool.tile([P, 2, T], f32, tag="t1a")
            for kc in range(2):
                nc.vector.tensor_tensor(t1a[:, kc], xq3[:, kc], Mb1,
                                        op=OP.subtract)
            u1 = apool.tile([P, 2, T], f32, tag="u1")
            for kc in range(2):
                nc.vector.scalar_tensor_tensor(u1[:, kc], t1a[:, kc],
                                               ln1w_c[:, kc:kc + 1], R1,
                                               op0=OP.mult, op1=OP.mult)
            nv_bf = apool.tile([P, 2, T], bf16, tag="nv_bf")
            for kc in range(2):
                nc.scalar.add(nv_bf[:, kc], u1[:, kc], ln1b_c[:, kc:kc + 1])

            # ---------------- LN2 mean, early: sum_f rv = sum_f nv
            #                  + colsum(Wp)@oT + sum bp  (before rv exists)
            nc.tensor.matmul(msum, ones_col[:], nv_bf[:, 0], start=True,
                             stop=False)
            nc.tensor.matmul(msum, ones_col[:], nv_bf[:, 1], start=False,
                             stop=False)
            nc.tensor.matmul(msum, wpcs[:, 0:1], oT_bf[:, 0], start=False,
                             stop=False)
            nc.tensor.matmul(msum, wpcs[:, 1:2], oT_bf[:, 1], start=False,
                             stop=True)
            m2 = apool.tile([1, T], f32, tag="m2")
            nc.vector.tensor_scalar(m2[:], msum, 1.0 / EMB, sumbp,
                                    op0=OP.mult, op1=OP.add)
            m2x2 = apool.tile([1, T], f32, tag="m2x2")
            nc.vector.tensor_tensor(m2x2[:], m2[:], m2[:], op=OP.mult)
            m2_bf = apool.tile([1, T], bf16, tag="m2_bf")
            nc.vector.tensor_copy(m2_bf[:], m2[:])
            nc.tensor.matmul(Mb2, ones_row[:], m2_bf[:], start=True, stop=True)

            # ---------------- projection + residual
            pp = ps_mm.tile([P, 512], f32, tag="mm")
            for mc in range(2):
                for kc in range(2):
                    nc.tensor.matmul(pp[:, mc * T:(mc + 1) * T],
                                     wp[:, kc, mc * P:(mc + 1) * P],
                                     oT_bf[:, kc], start=(kc == 0), stop=(kc == 1))
            rv = apool.tile([P, 2, T], f32, tag="rv")
            for mc in range(2):
                nc.vector.scalar_tensor_tensor(rv[:, mc], pp[:, mc * T:(mc + 1) * T],
                                               bp_c[:, mc:mc + 1], nv_bf[:, mc],
                                               op0=OP.add, op1=OP.add)

            # ---------------- LN2: stats after rv; apply via outer products
            t2 = apool.tile([P, 2, T], f32, tag="t2")
            for kc in range(2):
                nc.vector.tensor_tensor(t2[:, kc], rv[:, kc], Mb2,
                                        op=OP.subtract)
            sq2 = apool.tile([P, 2, T], bf16, tag="sq2")
            nc.scalar.square(sq2[:], rv[:])
            for kc in range(2):
                nc.tensor.matmul(ssq2, ones_col[:], sq2[:, kc],
                                 start=(kc == 0), stop=(kc == 1))
            d2 = apool.tile([1, T], f32, tag="d2")
            nc.vector.scalar_tensor_tensor(d2[:], ssq2, 1.0 / EMB, m2x2[:],
                                           op0=OP.mult, op1=OP.subtract)
            vstd2 = apool.tile([1, T], f32, tag="vstd2")
            nc.scalar.activation(vstd2[:], d2[:], AF.Sqrt, bias=epst[0:1, 0:1])
            r2 = apool.tile([1, T], f32, tag="r2")
            nc.vector.reciprocal_approx_fast(r2[:], vstd2[:])
            r2_bf = apool.tile([1, T], bf16, tag="r2_bf")
            nc.vector.tensor_copy(r2_bf[:], r2[:])
            nc.tensor.matmul(R2, ones_row[:], r2_bf[:], start=True, stop=True)
            u2 = apool.tile([P, 2, T], f32, tag="u2")
            for kc in range(2):
                nc.vector.scalar_tensor_tensor(u2[:, kc], t2[:, kc],
                                               ln2w_c[:, kc:kc + 1], R2,
                                               op0=OP.mult, op1=OP.mult)
            lv_bf = apool.tile([P, 2, T], bf16, tag="lv_bf")
            for kc in range(2):
                nc.scalar.add(lv_bf[:, kc], u2[:, kc], ln2b_c[:, kc:kc + 1])

            # ---------------- FFN + residual
            ph = ps_big.tile([P, 512], f32, tag="big")
            g_bf = apool.tile([P, 8, T], bf16, tag="gelu")
            for mc in range(8):
                for kc in range(2):
                    nc.tensor.matmul(ph[:, mc * T:(mc + 1) * T],
                                     w1[:, kc, mc * P:(mc + 1) * P],
                                     lv_bf[:, kc], start=(kc == 0), stop=(kc == 1))
                nc.scalar.activation(g_bf[:, mc], ph[:, mc * T:(mc + 1) * T],
                                     AF.Gelu, bias=b1_c[:, mc:mc + 1])

            out_sb = apool.tile([P, 2, T], f32, tag="out")
            out_r = out_d.rearrange("(c p) t -> p c t", p=P)
            for mc in range(2):
                pf = ps_mm.tile([P, 512], f32, tag="mm")
                for kc in range(8):
                    nc.tensor.matmul(pf[:, 0:T], w2[:, kc, mc * P:(mc + 1) * P],
                                     g_bf[:, kc], start=(kc == 0), stop=(kc == 7))
                nc.vector.scalar_tensor_tensor(out_sb[:, mc], pf[:, 0:T],
                                               b2_c[:, mc:mc + 1], lv_bf[:, mc],
                                               op0=OP.add, op1=OP.add)
                if mc == 0:
                    nc.sync.dma_start(out_r[:, mc], out_sb[:, mc])
                else:
                    nc.gpsimd.dma_start(out_r[:, mc], out_sb[:, mc])

    nc.compile()
    return nc


# ---------------------------------------------------------------- host side
def _reorder_qkv(W, b):
    W4 = np.asarray(W, np.float32).reshape(EMB, H, DH, 3)
    b4 = np.asarray(b, np.float32).reshape(H, DH, 3)
    return ([np.ascontiguousarray(W4[:, :, :, i].reshape(EMB, EMB)) for i in range(3)],
            [np.ascontiguousarray(b4[:, :, i].reshape(EMB)) for i in range(3)])


def _pack_w(w):
    """(K, M) f32 -> partition-major (128, K//128 * M) bf16 blob block."""
    w = np.asarray(w, np.float32)
    k, m = w.shape
    c = k // P
    return np.transpose(w.reshape(c, P, m), (1, 0, 2)).reshape(P, c * m).astype(BF)


def _pack_x(x):
    """(tokens, 256) -> (128, 2*tokens) f32 partition-major transposed."""
    xt = np.ascontiguousarray(np.asarray(x, np.float32).T)       # (256, t)
    t = xt.shape[1]
    return np.transpose(xt.reshape(2, P, t), (1, 0, 2)).reshape(P, 2 * t)


def _cols(v):
    """(256,) -> (128, 2) natural feature chunks."""
    return np.ascontiguousarray(np.asarray(v, np.float32).reshape(2, P).T)


def make_in_maps(inputs):
    inp = {k: np.asarray(v, np.float32) for k, v in inputs.items()}
    qkv_v = _reorder_qkv(inp['Wqkv_v'], inp['bqkv_v'])
    qkv_i = _reorder_qkv(inp['Wqkv_i'], inp['bqkv_i'])
    maps = []
    for core in range(NCORES):
        branch = core // 4
        r0 = (core % 4) * T
        if branch == 0:   # vis output: vis queries, ir keys/values
            x_own, x_oth = inp['vis_emb'][0], inp['ir_emb'][0]
            wq, bq = qkv_v[0][0], qkv_v[1][0]
            wk, bk = qkv_i[0][1], qkv_i[1][1]
            wv, bv = qkv_i[0][2], qkv_i[1][2]
            wp, bp = inp['Wp_v'], inp['bp_v']
            lnp = (inp['ln1v_w'], inp['ln1v_b'], inp['ln2v_w'], inp['ln2v_b'])
            w1, b1, w2, b2 = inp['W1v'], inp['b1v'], inp['W2v'], inp['b2v']
        else:             # ir output: ir queries, vis keys/values
            x_own, x_oth = inp['ir_emb'][0], inp['vis_emb'][0]
            wq, bq = qkv_i[0][0], qkv_i[1][0]
            wk, bk = qkv_v[0][1], qkv_v[1][1]
            wv, bv = qkv_v[0][2], qkv_v[1][2]
            wp, bp = inp['Wp_i'], inp['bp_i']
            lnp = (inp['ln1i_w'], inp['ln1i_b'], inp['ln2i_w'], inp['ln2i_b'])
            w1, b1, w2, b2 = inp['W1i'], inp['b1i'], inp['W2i'], inp['b2i']

        aux = np.zeros((P, 32), np.float32)
        aux[:, 0:2] = _cols(bq)
        aux[:, 2:4] = _cols(bk)
        aux[:, 4:6] = _cols(bp)
        aux[:, 6:8] = _cols(b2)
        aux[:, 8:16] = np.asarray(b1, np.float32).reshape(8, P).T
        aux[0, 16] = float(np.sum(np.asarray(bp, np.float32))) / EMB
        aux[:, 17:19] = _cols(lnp[1])
        aux[:, 19:21] = _cols(lnp[3])
        aux[:, 21:23] = _cols(lnp[0])
        aux[:, 23:25] = _cols(lnp[2])

        id98 = np.zeros((P, 98), BF)
        id98[0:98, 0:98] = np.eye(98, dtype=BF)

        wpcs = np.zeros((P, 2), BF)
        wpcs[:, :] = np.asarray(wp, np.float32).sum(axis=1).reshape(2, P).T.astype(BF)

        bv_rep = np.ascontiguousarray(np.broadcast_to(
            np.asarray(bv, np.float32).astype(BF)[None, :], (P, EMB)))

        blobA = np.concatenate([
            _pack_x(x_own[r0:r0 + T]),                  # 98
            aux,                                        # 32
            id98.view(np.float32),                      # 49
            _pack_w(wq).view(np.float32),               # 256
            wpcs.view(np.float32),                      # 1
            _pack_w(wk).view(np.float32),               # 256
            _pack_x(x_oth).astype(BF).view(np.float32),  # 196
        ], axis=1)
        blobB = np.concatenate([
            _pack_w(wv).view(np.float32),               # 256
            bv_rep.view(np.float32),                    # 128
            _pack_w(wp).view(np.float32),               # 256
        ], axis=1)
        maps.append({
            'blobA': np.ascontiguousarray(blobA),
            'blobB': np.ascontiguousarray(blobB),
            'blobC': np.ascontiguousarray(_pack_w(w1).view(np.float32)),
            'blobD': np.ascontiguousarray(_pack_w(w2).view(np.float32)),
        })
    return maps


def _recon(x):
    x = x.reshape(14, 14, 16, 16)
    x = np.transpose(x, (2, 3, 0, 1))
    return x.reshape(1, 1, 224, 224)


def assemble(core_outs):
    ov = np.concatenate([core_outs[c].T for c in range(4)], axis=0)
    oi = np.concatenate([core_outs[c].T for c in range(4, 8)], axis=0)
    return np.concatenate([_recon(oi), _recon(ov)], axis=1).astype(np.float32)


def get_nc():
    if 'nc' not in _CACHE:
        _CACHE['nc'] = build_bass()
    return _CACHE['nc']


def kernel(**inputs):
    from concourse import bass_utils
    nc = get_nc()
    in_maps = make_in_maps(inputs)
    res = bass_utils.run_bass_kernel_spmd(nc, in_maps, core_ids=list(range(NCORES)))
    outs = [np.asarray(r['out'], np.float32) for r in res.results]
    return assemble(outs)


# revision 35
# speedup vs baseline: 1.1570x; 1.1570x over previous
"""CrossViT fused block on 8 TRN2 NeuronCores.

Sharding: 2 branches (vis-output / ir-output) x 4-way token split -> 8 cores,
no collectives. Each core computes 49 output tokens of one branch end-to-end:
LN1, cross-attention (its queries vs all 196 keys/values of the other
modality), projection, residual, LN2, FFN, residual. Activations are kept
feature-major (features on SBUF partitions) so every linear layer is
matmul(lhsT=W_natural, rhs=xT). Matmul operands bf16 (fp32 PSUM).

vs. the first version: GpSimd is not used at all (its microcode library
swaps cost ~5us each); softmax normalization is a DVE tensor_scalar divide
by the per-partition exp-sum; LayerNorm mean/rstd are broadcast across
partitions with tiny PE outer-product matmuls against a ones row; ACT
tables (Exp/Sqrt/Gelu) are preloaded with dummy activations during the
input DMA; inputs arrive as 6 separately-DMA'd blobs ordered by first use;
q is computed densely then scattered lane-aligned into block-diag pair
tiles; k and attnV matmuls are pair-merged; LN2's mean is derived from
colsum(Wp) @ oT before the residual even lands.
"""
import sys
if '/opt/trn_rl_repo' not in sys.path:
    sys.path.insert(0, '/opt/trn_rl_repo')

import numpy as np
import ml_dtypes

BF = ml_dtypes.bfloat16
N, EMB, H, DH, HID = 196, 256, 8, 32, 1024
T = 49            # tokens per core
EPS, SCALE = 1e-5, 16.0
P = 128
NCORES = 8
TOKC = ((0, 128), (128, 68))   # token chunks of the 196 keys/values

# blobA f32 cols: [0:98] xq | [98:130] aux | [130:179] id98(bf16) |
#                 [179:435] wq(bf16) | [435:436] wpcs(bf16) |
#                 [436:692] wk(bf16) | [692:888] xkv(bf16)
CA = 888
# blobB: [0:256] wv(bf16) | [256:384] bv replicated (bf16) | [384:640] wp(bf16)
CB = 640
CC, CD = 1024, 1024   # w1, w2 (bf16)

_CACHE = {}


# ---------------------------------------------------------------- bass build
def build_bass():
    import concourse.bacc as bacc
    import concourse.mybir as mybir
    import concourse.tile as tile

    f32 = mybir.dt.float32
    bf16 = mybir.dt.bfloat16
    AF = mybir.ActivationFunctionType
    OP = mybir.AluOpType

    nc = bacc.Bacc("TRN2", target_bir_lowering=False)

    bA_d = nc.dram_tensor("blobA", [P, CA], f32, kind="ExternalInput")
    bB_d = nc.dram_tensor("blobB", [P, CB], f32, kind="ExternalInput")
    bC_d = nc.dram_tensor("blobC", [P, CC], f32, kind="ExternalInput")
    bD_d = nc.dram_tensor("blobD", [P, CD], f32, kind="ExternalInput")
    out_d = nc.dram_tensor("out", [EMB, T], f32, kind="ExternalOutput")

    with tile.TileContext(nc) as tc:
        with (
            tc.tile_pool(name="const", bufs=1) as cpool,
            tc.tile_pool(name="act", bufs=1) as apool,
            tc.tile_pool(name="attp", bufs=2) as attpool,
            tc.tile_pool(name="ps_mm", bufs=2, space="PSUM") as ps_mm,
            tc.tile_pool(name="ps_s", bufs=2, space="PSUM") as ps_s,
            tc.tile_pool(name="ps_t", bufs=2, space="PSUM") as ps_t,
            tc.tile_pool(name="ps_big", bufs=1, space="PSUM") as ps_big,
            tc.tile_pool(name="ps_ln", bufs=1, space="PSUM") as ps_ln,
        ):
            # ---- constants / scratch (no input deps; runs during DMA wait)
            ones_row = cpool.tile([1, P], bf16, tag="ones_row")
            nc.vector.memset(ones_row[:], 1.0)
            ones_col = cpool.tile([P, 1], bf16, tag="ones_col")
            nc.vector.memset(ones_col[:], 1.0)
            epst = cpool.tile([1, 1], f32, tag="epst")
            nc.vector.memset(epst[:], EPS)
            # block-diag q tiles: bdt[0] holds pairs 0,1 / bdt[1] pairs 2,3
            bdt = []
            for i in range(2):
                bd = apool.tile([P, 2 * T], bf16, tag=f"bd{i}")
                nc.vector.memset(bd[:], 0.0)
                bdt.append(bd)

            # ---- input DMAs, ordered by first use
            bA = cpool.tile([P, CA], f32, tag="bA")
            nc.sync.dma_start(bA[:], bA_d[:, :])
            bB = cpool.tile([P, CB], f32, tag="bB")
            nc.sync.dma_start(bB[:], bB_d[:, :])
            bC = cpool.tile([P, CC], f32, tag="bC")
            nc.sync.dma_start(bC[:], bC_d[:, :])
            bD = cpool.tile([P, CD], f32, tag="bD")
            nc.sync.dma_start(bD[:], bD_d[:, :])

            # ---- views
            xq3 = bA[:, 0:98].rearrange("p (c t) -> p c t", c=2)
            aux = bA[:, 98:130]
            id98 = bA[:, 130:179].bitcast(bf16)[0:98, 0:98]
            wq = bA[:, 179:435].bitcast(bf16).rearrange("p (c m) -> p c m", c=2)
            wpcs = bA[:, 435:436].bitcast(bf16)  # [128, 2]
            bq_c = aux[:, 0:2]
            bk_c = aux[:, 2:4]
            bp_c = aux[:, 4:6]
            b2_c = aux[:, 6:8]
            b1_c = aux[:, 8:16]
            sumbp = aux[0:1, 16:17]
            ln1b_c = aux[:, 17:19]
            ln2b_c = aux[:, 19:21]
            ln1w_c = aux[:, 21:23]
            ln2w_c = aux[:, 23:25]
            wk = bA[:, 436:692].bitcast(bf16).rearrange("p (c m) -> p c m", c=2)
            xkv = bA[:, 692:CA].bitcast(bf16).rearrange("p (c t) -> p c t", c=2)
            wv = bB[:, 0:256].bitcast(bf16).rearrange("p (c m) -> p c m", c=2)
            bvv = bB[:, 256:384].bitcast(bf16)
            wp = bB[:, 384:CB].bitcast(bf16).rearrange("p (c m) -> p c m", c=2)
            w1 = bC[:, 0:CC].bitcast(bf16).rearrange("p (c m) -> p c m", c=2)
            w2 = bD[:, 0:CD].bitcast(bf16).rearrange("p (c m) -> p c m", c=8)

            xq_bf = apool.tile([P, 2, T], bf16, tag="xq_bf")
            nc.vector.tensor_copy(xq_bf[:], xq3)

            # packed LN PSUM bank: col views
            ln = ps_ln.tile([P, 512], f32, tag="ln")
            lnp1 = ln[0:1, 0:98].rearrange("p (a t) -> p a t", a=2)
            msum = ln[0:1, 98:147]
            ssq2 = ln[0:1, 147:196]
            Mb1 = ln[:, 196:245]
            Mb2 = ln[:, 245:294]
            R1 = ln[:, 294:343]
            R2 = ln[:, 343:392]

            # ---------------- q dense (2 matmuls/chunk), lane-aligned scatter
            # into block-diag pair tiles; bias adds split DVE/ACT
            # (ACT Identity/Copy/Square live in every table -> no reload)
            for mc in range(2):
                qp = ps_mm.tile([P, 512], f32, tag="mm")
                for kc in range(2):
                    nc.tensor.matmul(qp[:, 0:T], wq[:, kc, mc * P:(mc + 1) * P],
                                     xq_bf[:, kc], start=(kc == 0), stop=(kc == 1))
                for hh in range(4):
                    j = hh % 2           # position within pair
                    r0 = hh * DH
                    nc.vector.tensor_scalar(
                        bdt[mc][r0:r0 + DH, j * T:(j + 1) * T],
                        qp[r0:r0 + DH, 0:T],
                        bq_c[r0:r0 + DH, mc:mc + 1], None, op0=OP.add)

            # ---------------- k pair-merged: [128, 196] per half (2 pairs)
            # pk tiles borrow the big/attT PSUM slots (their later users start
            # well after the k bias reads) so k doesn't WAR-wait on q's slots
            kt = []
            for i in range(2):
                pool = ps_big if i == 0 else ps_t
                pk = pool.tile([P, 512], f32, tag="big" if i == 0 else "attT")
                for kc in range(2):
                    nc.tensor.matmul(pk[:, 0:N], wk[:, kc, i * P:(i + 1) * P],
                                     xkv[:, kc], start=(kc == 0), stop=(kc == 1))
                k = apool.tile([P, N], bf16, tag=f"k{i}")
                nc.scalar.add(k[:], pk[:, 0:N], bk_c[:, i:i + 1])
                kt.append(k)

            # ---------------- v token-major
            v_bf = []
            for tcx, (t0, tsz) in enumerate(TOKC):
                pv = ps_mm.tile([P, 512], f32, tag="mm")
                for kc in range(2):
                    nc.tensor.matmul(pv[:tsz, 0:EMB], xkv[:, kc, t0:t0 + tsz],
                                     wv[:, kc], start=(kc == 0), stop=(kc == 1))
                vt = apool.tile([P, EMB], bf16, tag=f"v{tcx}")
                nc.vector.tensor_tensor(vt[:tsz], pv[:tsz, 0:EMB], bvv[0:tsz],
                                        op=OP.add)
                v_bf.append(vt)

            # ---------------- LN1 stats (off critical path)
            xsq1 = apool.tile([P, 2, 2, T], bf16, tag="xsq1")
            nc.vector.tensor_copy(xsq1[:, :, 0], xq_bf[:])
            nc.vector.tensor_tensor(xsq1[:, :, 1], xq_bf[:], xq_bf[:], op=OP.mult)
            for kc in range(2):
                nc.tensor.matmul(lnp1.rearrange("p a t -> p (a t)"), ones_col[:],
                                 xsq1[:, kc].rearrange("p a t -> p (a t)"),
                                 start=(kc == 0), stop=(kc == 1))

            # ---------------- scores + softmax (DVE reciprocal + scale)
            ssum = apool.tile([2 * T, 4], f32, tag="ssum")
            rsum = apool.tile([2 * T, 4], f32, tag="rsum")
            att_tiles = []
            for pr in range(4):
                prow = (pr % 2) * 64
                pss = ps_s.tile([2 * T, 512], f32, tag="scores")
                nc.tensor.matmul(pss[:, 0:N], bdt[pr // 2][prow:prow + 64, :],
                                 kt[pr // 2][prow:prow + 64, :],
                                 start=True, stop=True)
                atf = attpool.tile([2 * T, N], bf16, tag="attf")
                nc.scalar.activation(atf[:], pss[:, 0:N], AF.Exp, scale=1.0 / SCALE,
                                     accum_out=ssum[:, pr:pr + 1])
                nc.vector.reciprocal(rsum[:, pr:pr + 1], ssum[:, pr:pr + 1])
                att = attpool.tile([2 * T, N], bf16, tag="attn")
                nc.vector.tensor_scalar(att[:], atf[:], rsum[:, pr:pr + 1],
                                        None, op0=OP.mult)
                att_tiles.append(att)

            # ---------------- attT via PE transpose; pair-merged attnV
            oT_bf = apool.tile([P, 2, T], bf16, tag="oT_bf")
            sc = ps_big.tile([P, 512], f32, tag="big")
            for pr in range(4):
                prow = (pr % 2) * 64
                c0 = (pr // 2) * 2 * T
                ats = []
                for tcx, (t0, tsz) in enumerate(TOKC):
                    pt = ps_t.tile([P, 1024], bf16, tag="attT")
                    nc.tensor.transpose(pt[:tsz, 0:2 * T],
                                        att_tiles[pr][:, t0:t0 + tsz], id98[:])
                    at = attpool.tile([P, 2 * T], bf16, tag="atT")
                    nc.vector.tensor_copy(at[:tsz], pt[:tsz, 0:2 * T])
                    ats.append(at)
                for tcx, (t0, tsz) in enumerate(TOKC):
                    nc.tensor.matmul(sc[prow:prow + 64, c0:c0 + 2 * T],
                                     v_bf[tcx][:tsz, pr * 64:(pr + 1) * 64],
                                     ats[tcx][:tsz], start=(tcx == 0),
                                     stop=(tcx == 1), tile_position=(0, prow))
                nc.vector.tensor_copy(oT_bf[prow:prow + DH, pr // 2],
                                      sc[prow:prow + DH, c0:c0 + T])
                nc.vector.tensor_copy(oT_bf[prow + DH:prow + 64, pr // 2],
                                      sc[prow + DH:prow + 64, c0 + T:c0 + 2 * T])

            # ---------------- LN1 finish: mean/rstd + outer-product broadcast
            m1 = apool.tile([1, T], f32, tag="m1")
            nc.vector.tensor_scalar(m1[:], lnp1[0:1, 0], 1.0 / EMB, None,
                                    op0=OP.mult)
            # m2x1 bypass-reads rsum[:,3] so LN1's Sqrt lands after all 4 Exps
            # on the ACT queue (the ACT engine holds one table; interleaving
            # Sqrt between Exps costs a 1.3us table reload each way)
            m2x1 = apool.tile([1, T], f32, tag="m2x1")
            nc.vector.scalar_tensor_tensor(m2x1[:], m1[:], rsum[0:1, 3:4],
                                           m1[:], op0=OP.bypass, op1=OP.mult)
            d1 = apool.tile([1, T], f32, tag="d1")
            nc.vector.scalar_tensor_tensor(d1[:], lnp1[0:1, 1], 1.0 / EMB,
                                           m2x1[:], op0=OP.mult, op1=OP.subtract)
            vstd1 = apool.tile([1, T], f32, tag="vstd1")
            nc.scalar.activation(vstd1[:], d1[:], AF.Sqrt, bias=epst[0:1, 0:1])
            r1 = apool.tile([1, T], f32, tag="r1")
            nc.vector.reciprocal_approx_fast(r1[:], vstd1[:])
            mr1_bf = apool.tile([1, 2, T], bf16, tag="mr1_bf")
            nc.vector.tensor_copy(mr1_bf[0:1, 0], m1[:])
            nc.vector.tensor_copy(mr1_bf[0:1, 1], r1[:])
            nc.tensor.matmul(Mb1, ones_row[:], mr1_bf[0:1, 0], start=True,
                             stop=True)
            nc.tensor.matmul(R1, ones_row[:], mr1_bf[0:1, 1], start=True,
                             stop=True)
            t1a = apool.tile([P, 2, T], f32, tag="t1a")
            for kc in range(2):
                nc.vector.tensor_tensor(t1a[:, kc], xq3[:, kc], Mb1,
                                        op=OP.subtract)
            u1 = apool.tile([P, 2, T], f32, tag="u1")
            for kc in range(2):
                nc.vector.scalar_tensor_tensor(u1[:, kc], t1a[:, kc],
                                               ln1w_c[:, kc:kc + 1], R1,
                                               op0=OP.mult, op1=OP.mult)
            nv_bf = apool.tile([P, 2, T], bf16, tag="nv_bf")
            for kc in range(2):
                nc.scalar.add(nv_bf[:, kc], u1[:, kc], ln1b_c[:, kc:kc + 1])

            # ---------------- LN2 mean, early: sum_f rv = sum_f nv
            #                  + colsum(Wp)@oT + sum bp  (before rv exists)
            nc.tensor.matmul(msum, ones_col[:], nv_bf[:, 0], start=True,
                             stop=False)
            nc.tensor.matmul(msum, ones_col[:], nv_bf[:, 1], start=False,
                             stop=False)
            nc.tensor.matmul(msum, wpcs[:, 0:1], oT_bf[:, 0], start=False,
                             stop=False)
            nc.tensor.matmul(msum, wpcs[:, 1:2], oT_bf[:, 1], start=False,
                             stop=True)
            m2 = apool.tile([1, T], f32, tag="m2")
            nc.vector.tensor_scalar(m2[:], msum, 1.0 / EMB, sumbp,
                                    op0=OP.mult, op1=OP.add)
            m2x2 = apool.tile([1, T], f32, tag="m2x2")
            nc.vector.tensor_tensor(m2x2[:], m2[:], m2[:], op=OP.mult)
            m2_bf = apool.tile([1, T], bf16, tag="m2_bf")
            nc.vector.tensor_copy(m2_bf[:], m2[:])
            nc.tensor.matmul(Mb2, ones_row[:], m2_bf[:], start=True, stop=True)

            # ---------------- projection + residual
            pp = ps_mm.tile([P, 512], f32, tag="mm")
            for mc in range(2):
                for kc in range(2):
                    nc.tensor.matmul(pp[:, mc * T:(mc + 1) * T],
                                     wp[:, kc, mc * P:(mc + 1) * P],
                                     oT_bf[:, kc], start=(kc == 0), stop=(kc == 1))
            rv = apool.tile([P, 2, T], f32, tag="rv")
            for mc in range(2):
                nc.vector.scalar_tensor_tensor(rv[:, mc], pp[:, mc * T:(mc + 1) * T],
                                               bp_c[:, mc:mc + 1], nv_bf[:, mc],
                                               op0=OP.add, op1=OP.add)

            # ---------------- LN2: stats after rv; apply via outer products
            t2 = apool.tile([P, 2, T], f32, tag="t2")
            for kc in range(2):
                nc.vector.tensor_tensor(t2[:, kc], rv[:, kc], Mb2,
                                        op=OP.subtract)
            sq2 = apool.tile([P, 2, T], bf16, tag="sq2")
            nc.scalar.square(sq2[:], rv[:])
            for kc in range(2):
                nc.tensor.matmul(ssq2, ones_col[:], sq2[:, kc],
                                 start=(kc == 0), stop=(kc == 1))
            d2 = apool.tile([1, T], f32, tag="d2")
            nc.vector.scalar_tensor_tensor(d2[:], ssq2, 1.0 / EMB, m2x2[:],
                                           op0=OP.mult, op1=OP.subtract)
            vstd2 = apool.tile([1, T], f32, tag="vstd2")
            nc.scalar.activation(vstd2[:], d2[:], AF.Sqrt, bias=epst[0:1, 0:1])
            r2 = apool.tile([1, T], f32, tag="r2")
            nc.vector.reciprocal_approx_fast(r2[:], vstd2[:])
            r2_bf = apool.tile([1, T], bf16, tag="r2_bf")
            nc.vector.tensor_copy(r2_bf[:], r2[:])
            nc.tensor.matmul(R2, ones_row[:], r2_bf[:], start=True, stop=True)
            u2 = apool.tile([P, 2, T], f32, tag="u2")
            for kc in range(2):
                nc.vector.scalar_tensor_tensor(u2[:, kc], t2[:, kc],
                                               ln2w_c[:, kc:kc + 1], R2,
                                               op0=OP.mult, op1=OP.mult)
            lv_bf = apool.tile([P, 2, T], bf16, tag="lv_bf")
            for kc in range(2):
                nc.scalar.add(lv_bf[:, kc], u2[:, kc], ln2b_c[:, kc:kc + 1])

            # ---------------- FFN + residual
            ph = ps_big.tile([P, 512], f32, tag="big")
            g_bf = apool.tile([P, 8, T], bf16, tag="gelu")
            for mc in range(8):
                for kc in range(2):
                    nc.tensor.matmul(ph[:, mc * T:(mc + 1) * T],
                                     w1[:, kc, mc * P:(mc + 1) * P],
                                     lv_bf[:, kc], start=(kc == 0), stop=(kc == 1))
                nc.scalar.activation(g_bf[:, mc], ph[:, mc * T:(mc + 1) * T],
                                     AF.Gelu, bias=b1_c[:, mc:mc + 1])

            out_sb = apool.tile([P, 2, T], f32, tag="out")
            out_r = out_d.rearrange("(c p) t -> p c t", p=P)
            for mc in range(2):
                pf = ps_mm.tile([P, 512], f32, tag="mm")
                for kc in range(8):
                    nc.tensor.matmul(pf[:, 0:T], w2[:, kc, mc * P:(mc + 1) * P],
                                     g_bf[:, kc], start=(kc == 0), stop=(kc == 7))
                nc.vector.scalar_tensor_tensor(out_sb[:, mc], pf[:, 0:T],
                                               b2_c[:, mc:mc + 1], lv_bf[:, mc],
                                               op0=OP.add, op1=OP.add)
                if mc == 0:
                    nc.sync.dma_start(out_r[:, mc], out_sb[:, mc])
                else:
                    nc.gpsimd.dma_start(out_r[:, mc], out_sb[:, mc])

    nc.compile()
    return nc


# ---------------------------------------------------------------- host side
def _reorder_qkv(W, b):
    W4 = np.asarray(W, np.float32).reshape(EMB, H, DH, 3)
    b4 = np.asarray(b, np.float32).reshape(H, DH, 3)
    return ([np.ascontiguousarray(W4[:, :, :, i].reshape(EMB, EMB)) for i in range(3)],
            [np.ascontiguousarray(b4[:, :, i].reshape(EMB)) for i in range(3)])


def _pack_w(w):
    """(K, M) f32 -> partition-major (128, K//128 * M) bf16 blob block."""
    w = np.asarray(w, np.float32)
    k, m = w.shape
    c = k // P
    return np.transpose(w.reshape(c, P, m), (1, 0, 2)).reshape(P, c * m).astype(BF)


def _pack_x(x):
    """(tokens, 256) -> (128, 2*tokens) f32 partition-major transposed."""
    xt = np.ascontiguousarray(np.asarray(x, np.float32).T)       # (256, t)
    t = xt.shape[1]
    return np.transpose(xt.reshape(2, P, t), (1, 0, 2)).reshape(P, 2 * t)


def _cols(v):
    """(256,) -> (128, 2) natural feature chunks."""
    return np.ascontiguousarray(np.asarray(v, np.float32).reshape(2, P).T)


def make_in_maps(inputs):
    inp = {k: np.asarray(v, np.float32) for k, v in inputs.items()}
    qkv_v = _reorder_qkv(inp['Wqkv_v'], inp['bqkv_v'])
    qkv_i = _reorder_qkv(inp['Wqkv_i'], inp['bqkv_i'])
    maps = []
    for core in range(NCORES):
        branch = core // 4
        r0 = (core % 4) * T
        if branch == 0:   # vis output: vis queries, ir keys/values
            x_own, x_oth = inp['vis_emb'][0], inp['ir_emb'][0]
            wq, bq = qkv_v[0][0], qkv_v[1][0]
            wk, bk = qkv_i[0][1], qkv_i[1][1]
            wv, bv = qkv_i[0][2], qkv_i[1][2]
            wp, bp = inp['Wp_v'], inp['bp_v']
            lnp = (inp['ln1v_w'], inp['ln1v_b'], inp['ln2v_w'], inp['ln2v_b'])
            w1, b1, w2, b2 = inp['W1v'], inp['b1v'], inp['W2v'], inp['b2v']
        else:             # ir output: ir queries, vis keys/values
            x_own, x_oth = inp['ir_emb'][0], inp['vis_emb'][0]
            wq, bq = qkv_i[0][0], qkv_i[1][0]
            wk, bk = qkv_v[0][1], qkv_v[1][1]
            wv, bv = qkv_v[0][2], qkv_v[1][2]
            wp, bp = inp['Wp_i'], inp['bp_i']
            lnp = (inp['ln1i_w'], inp['ln1i_b'], inp['ln2i_w'], inp['ln2i_b'])
            w1, b1, w2, b2 = inp['W1i'], inp['b1i'], inp['W2i'], inp['b2i']

        aux = np.zeros((P, 32), np.float32)
        aux[:, 0:2] = _cols(bq)
        aux[:, 2:4] = _cols(bk)
        aux[:, 4:6] = _cols(bp)
        aux[:, 6:8] = _cols(b2)
        aux[:, 8:16] = np.asarray(b1, np.float32).reshape(8, P).T
        aux[0, 16] = float(np.sum(np.asarray(bp, np.float32))) / EMB
        aux[:, 17:19] = _cols(lnp[1])
        aux[:, 19:21] = _cols(lnp[3])
        aux[:, 21:23] = _cols(lnp[0])
        aux[:, 23:25] = _cols(lnp[2])

        id98 = np.zeros((P, 98), BF)
        id98[0:98, 0:98] = np.eye(98, dtype=BF)

        wpcs = np.zeros((P, 2), BF)
        wpcs[:, :] = np.asarray(wp, np.float32).sum(axis=1).reshape(2, P).T.astype(BF)

        bv_rep = np.ascontiguousarray(np.broadcast_to(
            np.asarray(bv, np.float32).astype(BF)[None, :], (P, EMB)))

        blobA = np.concatenate([
            _pack_x(x_own[r0:r0 + T]),                  # 98
            aux,                                        # 32
            id98.view(np.float32),                      # 49
            _pack_w(wq).view(np.float32),               # 256
            wpcs.view(np.float32),                      # 1
            _pack_w(wk).view(np.float32),               # 256
            _pack_x(x_oth).astype(BF).view(np.float32),  # 196
        ], axis=1)
        blobB = np.concatenate([
            _pack_w(wv).view(np.float32),               # 256
            bv_rep.view(np.float32),                    # 128
            _pack_w(wp).view(np.float32),               # 256
        ], axis=1)
        maps.append({
            'blobA': np.ascontiguousarray(blobA),
            'blobB': np.ascontiguousarray(blobB),
            'blobC': np.ascontiguousarray(_pack_w(w1).view(np.float32)),
            'blobD': np.ascontiguousarray(_pack_w(w2).view(np.float32)),
        })
    return maps


def _recon(x):
    x = x.reshape(14, 14, 16, 16)
    x = np.transpose(x, (2, 3, 0, 1))
    return x.reshape(1, 1, 224, 224)


def assemble(core_outs):
    ov = np.concatenate([core_outs[c].T for c in range(4)], axis=0)
    oi = np.concatenate([core_outs[c].T for c in range(4, 8)], axis=0)
    return np.concatenate([_recon(oi), _recon(ov)], axis=1).astype(np.float32)


def get_nc():
    if 'nc' not in _CACHE:
        _CACHE['nc'] = build_bass()
    return _CACHE['nc']


def kernel(**inputs):
    from concourse import bass_utils
    nc = get_nc()
    in_maps = make_in_maps(inputs)
    res = bass_utils.run_bass_kernel_spmd(nc, in_maps, core_ids=list(range(NCORES)))
    outs = [np.asarray(r['out'], np.float32) for r in res.results]
    return assemble(outs)


# revision 64
# speedup vs baseline: 1.1845x; 1.0238x over previous
"""CrossViT fused block on 8 TRN2 NeuronCores.

Sharding: 2 branches (vis-output / ir-output) x 4-way token split -> 8 cores,
no collectives. Each core computes 49 output tokens of one branch end-to-end:
LN1, cross-attention (its queries vs all 196 keys/values of the other
modality), projection, residual, LN2, FFN, residual. Activations are kept
feature-major (features on SBUF partitions) so every linear layer is
matmul(lhsT=W_natural, rhs=xT).

Engine plan: no GpSimd (its microcode library swaps cost ~5us each).
Weight matmuls (q/k/v/proj/FFN) run in fp8e4 DoubleRow perf mode: both
128-row K-tiles of the K=256 contraction in one PE instruction. Weights are
scaled x32 on the host to dodge e4m3 denormals; the 1/32 rides existing
bias/scale slots (tensor_scalar, activation scale). Scores/attnV/transposes
stay bf16. Softmax normalization is a DVE tensor_scalar multiply by the
reciprocal exp-sum; LayerNorm mean/rstd broadcasts are tiny PE outer
products against a ones row; LN2's mean comes from colsum(Wp) @ oT before
the residual lands. ACT does only Exp/Sqrt/Gelu plus table-free
Identity/Square offloads.
"""
import sys
if '/opt/trn_rl_repo' not in sys.path:
    sys.path.insert(0, '/opt/trn_rl_repo')

import numpy as np
import ml_dtypes

BF = ml_dtypes.bfloat16
F8 = ml_dtypes.float8_e4m3fn
N, EMB, H, DH, HID = 196, 256, 8, 32, 1024
T = 49            # tokens per core
EPS, SCALE = 1e-5, 16.0
WS = 32.0         # host-side fp8 weight scale
P = 128
NCORES = 8
TOKC = ((0, 128), (128, 68))   # token chunks of the 196 keys/values

# blobA f32 cols: [0:98] xq f32 | [98:130] aux | [130:179] id98(bf16) |
#   [179:204] xq(f8) | [204:332] wq(f8i) | [332:460] wk(f8i) |
#   [460:558] xkv(f8, k rhs) | [558:686] xkv(f8i, v lhsT, 2x128 tokens) |
#   [686:687] wpcs(f8)
CA = 687
# blobB: [0:128] wv(f8) | [128:256] bv replicated(bf16) | [256:384] wp(f8i)
CB = 384
# blobC: [0:512] w1(f8i) | [512:1024] w2(f8i)  (bf16 fallback: 2048 cols)
import os
FFN_BF16 = bool(int(os.environ.get('FFN_BF16', '0')))
CC = 2048 if FFN_BF16 else 1024

_CACHE = {}


# ---------------------------------------------------------------- bass build
def build_bass():
    import concourse.bacc as bacc
    import concourse.mybir as mybir
    import concourse.tile as tile

    f32 = mybir.dt.float32
    bf16 = mybir.dt.bfloat16
    f8 = mybir.dt.float8e4
    AF = mybir.ActivationFunctionType
    OP = mybir.AluOpType
    DR = mybir.MatmulPerfMode.DoubleRowSwInterleave

    nc = bacc.Bacc("TRN2", target_bir_lowering=False)

    bA_d = nc.dram_tensor("blobA", [P, CA], f32, kind="ExternalInput")
    bB_d = nc.dram_tensor("blobB", [P, CB], f32, kind="ExternalInput")
    bC_d = nc.dram_tensor("blobC", [P, CC], f32, kind="ExternalInput")
    out_d = nc.dram_tensor("out", [EMB, T], f32, kind="ExternalOutput")

    with tile.TileContext(nc) as tc:
        with (
            tc.tile_pool(name="const", bufs=1) as cpool,
            tc.tile_pool(name="act", bufs=1) as apool,
            tc.tile_pool(name="attp", bufs=2) as attpool,
            tc.tile_pool(name="ps_mm", bufs=2, space="PSUM") as ps_mm,
            tc.tile_pool(name="ps_s", bufs=2, space="PSUM") as ps_s,
            tc.tile_pool(name="ps_t", bufs=2, space="PSUM") as ps_t,
            tc.tile_pool(name="ps_big", bufs=1, space="PSUM") as ps_big,
            tc.tile_pool(name="ps_ln", bufs=1, space="PSUM") as ps_ln,
        ):
            # ---- constants / scratch (no input deps; runs during DMA wait)
            ones_row = cpool.tile([1, P], bf16, tag="ones_row")
            nc.vector.memset(ones_row[:], 1.0)
            ones_col = cpool.tile([P, 1], bf16, tag="ones_col")
            nc.vector.memset(ones_col[:], 1.0)
            epst = cpool.tile([1, 1], f32, tag="epst")
            nc.vector.memset(epst[:], EPS)
            # block-diag q tiles: bdt[0] holds pairs 0,1 / bdt[1] pairs 2,3
            bdt = []
            for i in range(2):
                bd = apool.tile([P, 2 * T], bf16, tag=f"bd{i}")
                nc.vector.memset(bd[:], 0.0)
                bdt.append(bd)

            # ---- input DMAs, ordered by first use
            bA = cpool.tile([P, CA], f32, tag="bA")
            nc.sync.dma_start(bA[:], bA_d[:, :])
            bB = cpool.tile([P, CB], f32, tag="bB")
            nc.sync.dma_start(bB[:], bB_d[:, :])
            bC = cpool.tile([P, CC], f32, tag="bC")
            nc.sync.dma_start(bC[:], bC_d[:, :])

            # ---- views
            xq3 = bA[:, 0:98].rearrange("p (c t) -> p c t", c=2)
            aux = bA[:, 98:130]
            id98 = bA[:, 130:179].bitcast(bf16)[0:98, 0:98]
            xq8 = bA[:, 179:204].bitcast(f8)[:, 0:98].rearrange(
                "p (c t) -> p c t", c=2)
            wq = bA[:, 204:332].bitcast(f8).rearrange("p (c m) -> p c m", c=2)
            wk = bA[:, 332:460].bitcast(f8).rearrange("p (c m) -> p c m", c=2)
            xkv = bA[:, 460:558].bitcast(f8).rearrange("p (c t) -> p c t", c=2)
            xkvi = bA[:, 558:686].bitcast(f8)     # [128, 512] interleaved/padded
            wpcs = bA[:, 686:687].bitcast(f8)   # [128, 4]: kc0@0, kc1@2
            bq_c = aux[:, 0:2]
            bk_c = aux[:, 2:4]
            b2_c = aux[:, 6:8]
            b1_c = aux[:, 8:16]
            sumbp = aux[0:1, 16:17]
            ln1bp_c = aux[:, 17:19]    # ln1b + bp (for rv via nv')
            ln2b_c = aux[:, 19:21]
            ln1w_c = aux[:, 21:23]
            ln2w_c = aux[:, 23:25]
            ln2bb2_c = aux[:, 25:27]   # ln2b + b2 (for residual via lvb2)
            wv = bB[:, 0:128].bitcast(f8).rearrange("p (c m) -> p c m", c=2)
            bvv = bB[:, 128:256].bitcast(bf16)
            wp = bB[:, 256:CB].bitcast(f8).rearrange("p (b w) -> p b w", b=2)
            if FFN_BF16:
                w1 = bC[:, 0:1024].bitcast(bf16).rearrange(
                    "p (c m) -> p c m", c=2)
                w2 = bC[:, 1024:CC].bitcast(bf16).rearrange(
                    "p (c m) -> p c m", c=8)
            else:
                w1 = bC[:, 0:512].bitcast(f8).rearrange(
                    "p (b w) -> p b w", b=8)
                w2 = bC[:, 512:CC].bitcast(f8).rearrange(
                    "p (b i w) -> p b i w", b=2, i=4)

            xq_bf = apool.tile([P, 2, T], bf16, tag="xq_bf")
            nc.vector.tensor_copy(xq_bf[:], xq3)

            # packed LN PSUM bank: col views
            ln = ps_ln.tile([P, 512], f32, tag="ln")
            lnp1 = ln[0:1, 0:98].rearrange("p (a t) -> p a t", a=2)
            msum = ln[0:1, 98:147]
            ssq2 = ln[0:1, 147:196]
            Mb1 = ln[:, 196:245]
            Mb2 = ln[:, 245:294]
            R1 = ln[:, 294:343]
            R2 = ln[:, 343:392]

            # ---------------- q dense fp8 DoubleRow (1 matmul/chunk),
            # lane-aligned scatter into block-diag pair tiles (undoes the x32)
            wq_i = wq.rearrange("p c m -> p (c m)").rearrange(
                "p (b w) -> p b w", b=2)
            for mc in range(2):
                qp = ps_mm.tile([P, 512], f32, tag="mm")
                nc.tensor.matmul(qp[:, 0:T], wq_i[:, mc], xq8[:],
                                 start=True, stop=True, perf_mode=DR)
                for hh in range(4):
                    j = hh % 2           # position within pair
                    r0 = hh * DH
                    nc.vector.tensor_scalar(
                        bdt[mc][r0:r0 + DH, j * T:(j + 1) * T],
                        qp[r0:r0 + DH, 0:T], 1.0 / WS,
                        bq_c[r0:r0 + DH, mc:mc + 1],
                        op0=OP.mult, op1=OP.add)

            # ---------------- k pair-merged fp8 DoubleRow: [128, 196]/half
            # pk tiles borrow the big/attT PSUM slots (their later users start
            # well after the k bias reads) so k doesn't WAR-wait on q's slots
            wk_i = wk.rearrange("p c m -> p (c m)").rearrange(
                "p (b w) -> p b w", b=2)
            kt = []
            for i in range(2):
                pool = ps_big if i == 0 else ps_t
                pk = pool.tile([P, 512], f32, tag="big" if i == 0 else "attT")
                nc.tensor.matmul(pk[:, 0:N], wk_i[:, i], xkv[:],
                                 start=True, stop=True, perf_mode=DR)
                k = apool.tile([P, N], bf16, tag=f"k{i}")
                nc.scalar.activation(k[:], pk[:, 0:N], AF.Identity,
                                     scale=1.0 / WS, bias=bk_c[:, i:i + 1])
                kt.append(k)

            # ---------------- v token-major fp8 DoubleRow
            v_bf = []
            for tcx, (t0, tsz) in enumerate(TOKC):
                pv = ps_mm.tile([P, 512], f32, tag="mm")
                nc.tensor.matmul(pv[:, 0:EMB],
                                 xkvi[:, 256 * tcx:256 * (tcx + 1)],
                                 wv[:], start=True, stop=True, perf_mode=DR)
                vt = apool.tile([P, EMB], bf16, tag=f"v{tcx}")
                nc.vector.scalar_tensor_tensor(vt[:tsz], pv[:tsz, 0:EMB],
                                               1.0 / WS, bvv[0:tsz],
                                               op0=OP.mult, op1=OP.add)
                v_bf.append(vt)

            # ---------------- LN1 stats (off critical path)
            xsq1 = apool.tile([P, 2, 2, T], bf16, tag="xsq1")
            nc.vector.tensor_copy(xsq1[:, :, 0], xq_bf[:])
            nc.vector.tensor_tensor(xsq1[:, :, 1], xq_bf[:], xq_bf[:], op=OP.mult)
            for kc in range(2):
                nc.tensor.matmul(lnp1.rearrange("p a t -> p (a t)"), ones_col[:],
                                 xsq1[:, kc].rearrange("p a t -> p (a t)"),
                                 start=(kc == 0), stop=(kc == 1))

            # ---------------- scores + softmax (recip + scale on DVE)
            ssum = apool.tile([2 * T, 4], f32, tag="ssum")
            rsum = apool.tile([2 * T, 4], f32, tag="rsum")
            att_tiles = []
            for pr in range(4):
                prow = (pr % 2) * 64
                pss = ps_s.tile([2 * T, 512], f32, tag="scores")
                nc.tensor.matmul(pss[:, 0:N], bdt[pr // 2][prow:prow + 64, :],
                                 kt[pr // 2][prow:prow + 64, :],
                                 start=True, stop=True)
                atf = attpool.tile([2 * T, N], bf16, tag="attf")
                nc.scalar.activation(atf[:], pss[:, 0:N], AF.Exp, scale=1.0 / SCALE,
                                     accum_out=ssum[:, pr:pr + 1])
                nc.vector.reciprocal(rsum[:, pr:pr + 1], ssum[:, pr:pr + 1])
                att = attpool.tile([2 * T, N], bf16, tag="attn")
                nc.vector.tensor_scalar(att[:], atf[:], rsum[:, pr:pr + 1],
                                        None, op0=OP.mult)
                att_tiles.append(att)

            # ---------------- attT via PE transpose; pair-merged attnV
            oT8 = apool.tile([P, 2, T], f8, tag="oT8")
            sc = ps_big.tile([P, 512], f32, tag="big")
            for pr in range(4):
                prow = (pr % 2) * 64
                c0 = (pr // 2) * 2 * T
                ats = []
                for tcx, (t0, tsz) in enumerate(TOKC):
                    pt = ps_t.tile([P, 1024], bf16, tag="attT")
                    nc.tensor.transpose(pt[:tsz, 0:2 * T],
                                        att_tiles[pr][:, t0:t0 + tsz], id98[:])
                    at = attpool.tile([P, 2 * T], bf16, tag="atT")
                    nc.vector.tensor_copy(at[:tsz], pt[:tsz, 0:2 * T])
                    ats.append(at)
                for tcx, (t0, tsz) in enumerate(TOKC):
                    nc.tensor.matmul(sc[prow:prow + 64, c0:c0 + 2 * T],
                                     v_bf[tcx][:tsz, pr * 64:(pr + 1) * 64],
                                     ats[tcx][:tsz], start=(tcx == 0),
                                     stop=(tcx == 1), tile_position=(0, prow))
                nc.vector.tensor_copy(oT8[prow:prow + DH, pr // 2],
                                      sc[prow:prow + DH, c0:c0 + T])
                nc.vector.tensor_copy(oT8[prow + DH:prow + 64, pr // 2],
                                      sc[prow + DH:prow + 64, c0 + T:c0 + 2 * T])

            # ---------------- LN1 finish: mean/rstd + outer-product broadcast
            m1 = apool.tile([1, T], f32, tag="m1")
            nc.vector.tensor_scalar(m1[:], lnp1[0:1, 0], 1.0 / EMB, None,
                                    op0=OP.mult)
            m2x1 = apool.tile([1, T], f32, tag="m2x1")
            nc.vector.tensor_tensor(m2x1[:], m1[:], m1[:], op=OP.mult)
            d1 = apool.tile([1, T], f32, tag="d1")
            nc.vector.scalar_tensor_tensor(d1[:], lnp1[0:1, 1], 1.0 / EMB,
                                           m2x1[:], op0=OP.mult, op1=OP.subtract)
            vstd1 = apool.tile([1, T], f32, tag="vstd1")
            nc.scalar.activation(vstd1[:], d1[:], AF.Sqrt, bias=epst[0:1, 0:1])
            r1 = apool.tile([1, T], f32, tag="r1")
            nc.vector.reciprocal_approx_fast(r1[:], vstd1[:])
            mr1_bf = apool.tile([1, 2, T], bf16, tag="mr1_bf")
            nc.vector.tensor_copy(mr1_bf[0:1, 0], m1[:])
            nc.vector.tensor_copy(mr1_bf[0:1, 1], r1[:])
            nc.tensor.matmul(Mb1, ones_row[:], mr1_bf[0:1, 0], start=True,
                             stop=True)
            nc.tensor.matmul(R1, ones_row[:], mr1_bf[0:1, 1], start=True,
                             stop=True)
            t1a = apool.tile([P, 2, T], f32, tag="t1a")
            for kc in range(2):
                nc.vector.tensor_tensor(t1a[:, kc], xq3[:, kc], Mb1,
                                        op=OP.subtract)
            u1 = apool.tile([P, 2, T], f32, tag="u1")
            for kc in range(2):
                nc.vector.scalar_tensor_tensor(u1[:, kc], t1a[:, kc],
                                               ln1w_c[:, kc:kc + 1], R1,
                                               op0=OP.mult, op1=OP.mult)
            # nv' = nv + bp (bp folded in so rv = pp/32 + nv')
            nv_bf = apool.tile([P, 2, T], bf16, tag="nv_bf")
            for kc in range(2):
                nc.scalar.add(nv_bf[:, kc], u1[:, kc], ln1bp_c[:, kc:kc + 1])

            # ---------------- LN2 mean, early: sum_f rv = sum_f nv'
            #                  + colsum(Wp)@oT/32  (before rv exists)
            nc.tensor.matmul(msum, ones_col[:], nv_bf[:, 0], start=True,
                             stop=False)
            nc.tensor.matmul(msum, ones_col[:], nv_bf[:, 1], start=False,
                             stop=False)
            for kc in range(2):
                nc.tensor.matmul(msum, wpcs[:, 2 * kc:2 * kc + 1], oT8[:, kc],
                                 start=False, stop=(kc == 1))
            m2 = apool.tile([1, T], f32, tag="m2")
            nc.vector.tensor_scalar(m2[:], msum, 1.0 / EMB, None, op0=OP.mult)
            m2x2 = apool.tile([1, T], f32, tag="m2x2")
            nc.vector.tensor_tensor(m2x2[:], m2[:], m2[:], op=OP.mult)
            m2_bf = apool.tile([1, T], bf16, tag="m2_bf")
            nc.vector.tensor_copy(m2_bf[:], m2[:])
            nc.tensor.matmul(Mb2, ones_row[:], m2_bf[:], start=True, stop=True)

            # ---------------- projection fp8 DoubleRow + residual
            pp = ps_mm.tile([P, 512], f32, tag="mm")
            for mc in range(2):
                # DR psum dst needs >=8B column alignment -> 64-col slots
                nc.tensor.matmul(pp[:, mc * 64:mc * 64 + T], wp[:, mc],
                                 oT8[:], start=True, stop=True, perf_mode=DR)
            rv = apool.tile([P, 2, T], f32, tag="rv")
            for mc in range(2):
                nc.vector.scalar_tensor_tensor(rv[:, mc], pp[:, mc * 64:mc * 64 + T],
                                               1.0 / WS, nv_bf[:, mc],
                                               op0=OP.mult, op1=OP.add)

            # ---------------- LN2: stats after rv; apply via outer products
            t2 = apool.tile([P, 2, T], f32, tag="t2")
            for kc in range(2):
                nc.vector.tensor_tensor(t2[:, kc], rv[:, kc], Mb2,
                                        op=OP.subtract)
            sq2 = apool.tile([P, 2, T], bf16, tag="sq2")
            nc.scalar.square(sq2[:], rv[:])
            for kc in range(2):
                nc.tensor.matmul(ssq2, ones_col[:], sq2[:, kc],
                                 start=(kc == 0), stop=(kc == 1))
            d2 = apool.tile([1, T], f32, tag="d2")
            nc.vector.scalar_tensor_tensor(d2[:], ssq2, 1.0 / EMB, m2x2[:],
                                           op0=OP.mult, op1=OP.subtract)
            vstd2 = apool.tile([1, T], f32, tag="vstd2")
            nc.scalar.activation(vstd2[:], d2[:], AF.Sqrt, bias=epst[0:1, 0:1])
            r2 = apool.tile([1, T], f32, tag="r2")
            nc.vector.reciprocal_approx_fast(r2[:], vstd2[:])
            r2_bf = apool.tile([1, T], bf16, tag="r2_bf")
            nc.vector.tensor_copy(r2_bf[:], r2[:])
            nc.tensor.matmul(R2, ones_row[:], r2_bf[:], start=True, stop=True)
            u2 = apool.tile([P, 2, T], f32, tag="u2")
            for kc in range(2):
                nc.vector.scalar_tensor_tensor(u2[:, kc], t2[:, kc],
                                               ln2w_c[:, kc:kc + 1], R2,
                                               op0=OP.mult, op1=OP.mult)
            # lv for the FFN (fp8, +ln2b) on DVE; lv+b2 for the residual on ACT
            lv8 = apool.tile([P, 2, T], bf16 if FFN_BF16 else f8, tag="lv8")
            for kc in range(2):
                nc.vector.tensor_scalar(lv8[:, kc], u2[:, kc],
                                        ln2b_c[:, kc:kc + 1], None, op0=OP.add)
            lvb2 = apool.tile([P, 2, T], bf16, tag="lvb2")
            for kc in range(2):
                nc.scalar.add(lvb2[:, kc], u2[:, kc], ln2bb2_c[:, kc:kc + 1])

            # ---------------- FFN fp8 DoubleRow + residual
            ph = ps_big.tile([P, 512], f32, tag="big")
            g8 = apool.tile([P, 8, T], bf16 if FFN_BF16 else f8, tag="gelu")
            gf = AF.Identity if _CACHE.get('sim_ident_gelu') else AF.Gelu
            for mc in range(8):
                if FFN_BF16:
                    for kc in range(2):
                        nc.tensor.matmul(ph[:, mc * T:(mc + 1) * T],
                                         w1[:, kc, mc * P:(mc + 1) * P],
                                         lv8[:, kc], start=(kc == 0),
                                         stop=(kc == 1))
                    nc.scalar.activation(g8[:, mc], ph[:, mc * T:(mc + 1) * T],
                                         gf, bias=b1_c[:, mc:mc + 1])
                else:
                    nc.tensor.matmul(ph[:, mc * 64:mc * 64 + T], w1[:, mc],
                                     lv8[:], start=True, stop=True, perf_mode=DR)
                    nc.scalar.activation(g8[:, mc], ph[:, mc * 64:mc * 64 + T],
                                         gf, scale=1.0 / WS,
                                         bias=b1_c[:, mc:mc + 1])

            out_sb = apool.tile([P, 2, T], f32, tag="out")
            out_r = out_d.rearrange("(c p) t -> p c t", p=P)
            for mc in range(2):
                pf = ps_mm.tile([P, 512], f32, tag="mm")
                if FFN_BF16:
                    for kc in range(8):
                        nc.tensor.matmul(pf[:, 0:T],
                                         w2[:, kc, mc * P:(mc + 1) * P],
                                         g8[:, kc], start=(kc == 0),
                                         stop=(kc == 7))
                    nc.vector.scalar_tensor_tensor(out_sb[:, mc], pf[:, 0:T],
                                                   1.0, lvb2[:, mc],
                                                   op0=OP.mult, op1=OP.add)
                else:
                    for i in range(4):
                        nc.tensor.matmul(pf[:, 0:T], w2[:, mc, i],
                                         g8[:, 2 * i:2 * i + 2], start=(i == 0),
                                         stop=(i == 3), perf_mode=DR)
                    nc.vector.scalar_tensor_tensor(out_sb[:, mc], pf[:, 0:T],
                                                   1.0 / WS, lvb2[:, mc],
                                                   op0=OP.mult, op1=OP.add)
                if mc == 0:
                    nc.sync.dma_start(out_r[:, mc], out_sb[:, mc])
                else:
                    nc.gpsimd.dma_start(out_r[:, mc], out_sb[:, mc])

    nc.compile()
    return nc


# ---------------------------------------------------------------- host side
def _reorder_qkv(W, b):
    W4 = np.asarray(W, np.float32).reshape(EMB, H, DH, 3)
    b4 = np.asarray(b, np.float32).reshape(H, DH, 3)
    return ([np.ascontiguousarray(W4[:, :, :, i].reshape(EMB, EMB)) for i in range(3)],
            [np.ascontiguousarray(b4[:, :, i].reshape(EMB)) for i in range(3)])


def _pack_wbf(w):
    w = np.asarray(w, np.float32)
    k, m = w.shape
    c = k // P
    return np.transpose(w.reshape(c, P, m), (1, 0, 2)).reshape(P, c * m).astype(BF)


def _pack_w8(w):
    """(K, M) f32 -> x32-scaled partition-major (128, K//128 * M) f8 block."""
    w = np.asarray(w, np.float32) * WS
    k, m = w.shape
    c = k // P
    return np.transpose(w.reshape(c, P, m), (1, 0, 2)).reshape(P, c * m).astype(F8)


def _ileave(pair3):
    """[P, 2, m] -> [P, 2m]: A/B column-interleaved, columns reversed
    (DoubleRowSwInterleave weight layout)."""
    blk = pair3[:, :, ::-1]
    return np.ascontiguousarray(np.transpose(blk, (0, 2, 1)).reshape(P, -1))


def _pack_w8i(w, mc_cols, mc_major=False):
    """(K, M) f32 -> x32-scaled SwInterleave layout: one [P, 2*mc_cols]
    interleaved block per (k-tile pair, M-chunk)."""
    w = np.asarray(w, np.float32) * WS
    k, m = w.shape
    c = k // P
    w3 = np.transpose(w.reshape(c, P, m), (1, 0, 2)).astype(F8)  # [P, c, m]
    blk = [[_ileave(w3[:, 2 * i:2 * i + 2, j0:j0 + mc_cols])
            for j0 in range(0, m, mc_cols)] for i in range(c // 2)]
    if mc_major:
        out = [blk[i][j] for j in range(len(blk[0])) for i in range(len(blk))]
    else:
        out = [blk[i][j] for i in range(len(blk)) for j in range(len(blk[0]))]
    return np.concatenate(out, axis=1)


def _pack_x(x):
    """(tokens, 256) -> (128, 2*tokens) f32 partition-major transposed."""
    xt = np.ascontiguousarray(np.asarray(x, np.float32).T)       # (256, t)
    t = xt.shape[1]
    return np.transpose(xt.reshape(2, P, t), (1, 0, 2)).reshape(P, 2 * t)


def _f8cols(a):
    """(128, n) f8 array -> zero-padded f32-col view (n_f32 = ceil(n/4))."""
    n = a.shape[1]
    pad = (-n) % 4
    if pad:
        a = np.concatenate([a, np.zeros((P, pad), F8)], axis=1)
    return np.ascontiguousarray(a).view(np.float32)


def _cols(v):
    """(256,) -> (128, 2) natural feature chunks."""
    return np.ascontiguousarray(np.asarray(v, np.float32).reshape(2, P).T)


def make_in_maps(inputs):
    inp = {k: np.asarray(v, np.float32) for k, v in inputs.items()}
    qkv_v = _reorder_qkv(inp['Wqkv_v'], inp['bqkv_v'])
    qkv_i = _reorder_qkv(inp['Wqkv_i'], inp['bqkv_i'])
    maps = []
    for core in range(NCORES):
        branch = core // 4
        r0 = (core % 4) * T
        if branch == 0:   # vis output: vis queries, ir keys/values
            x_own, x_oth = inp['vis_emb'][0], inp['ir_emb'][0]
            wq, bq = qkv_v[0][0], qkv_v[1][0]
            wk, bk = qkv_i[0][1], qkv_i[1][1]
            wv, bv = qkv_i[0][2], qkv_i[1][2]
            wp, bp = inp['Wp_v'], inp['bp_v']
            lnp = (inp['ln1v_w'], inp['ln1v_b'], inp['ln2v_w'], inp['ln2v_b'])
            w1, b1, w2, b2 = inp['W1v'], inp['b1v'], inp['W2v'], inp['b2v']
        else:             # ir output: ir queries, vis kv
            x_own, x_oth = inp['ir_emb'][0], inp['vis_emb'][0]
            wq, bq = qkv_i[0][0], qkv_i[1][0]
            wk, bk = qkv_v[0][1], qkv_v[1][1]
            wv, bv = qkv_v[0][2], qkv_v[1][2]
            wp, bp = inp['Wp_i'], inp['bp_i']
            lnp = (inp['ln1i_w'], inp['ln1i_b'], inp['ln2i_w'], inp['ln2i_b'])
            w1, b1, w2, b2 = inp['W1i'], inp['b1i'], inp['W2i'], inp['b2i']

        aux = np.zeros((P, 32), np.float32)
        aux[:, 0:2] = _cols(bq)
        aux[:, 2:4] = _cols(bk)
        aux[:, 4:6] = _cols(bp)
        aux[:, 6:8] = _cols(b2)
        aux[:, 8:16] = np.asarray(b1, np.float32).reshape(8, P).T
        aux[0, 16] = float(np.sum(np.asarray(bp, np.float32))) / EMB
        aux[:, 17:19] = _cols(np.asarray(lnp[1], np.float32)
                              + np.asarray(bp, np.float32))
        aux[:, 19:21] = _cols(lnp[3])
        aux[:, 21:23] = _cols(lnp[0])
        aux[:, 23:25] = _cols(lnp[2])
        aux[:, 25:27] = _cols(np.asarray(lnp[3], np.float32)
                              + np.asarray(b2, np.float32))

        id98 = np.zeros((P, 98), BF)
        id98[0:98, 0:98] = np.eye(98, dtype=BF)

        xq_f = _pack_x(x_own[r0:r0 + T])
        xkv_f = _pack_x(x_oth)
        xkv3 = np.zeros((P, 2, 256), F8)
        xkv3[:, :, 0:N] = xkv_f.astype(F8).reshape(P, 2, N)
        xkvi = np.concatenate(
            [_ileave(xkv3[:, :, 0:128]), _ileave(xkv3[:, :, 128:256])], axis=1)
        wpcs = np.zeros((P, 4), F8)
        wpcs[:, 0:4:2] = np.asarray(
            wp, np.float32).sum(axis=1).reshape(2, P).T.astype(F8)

        bv_rep = np.ascontiguousarray(np.broadcast_to(
            np.asarray(bv, np.float32).astype(BF)[None, :], (P, EMB)))

        blobA = np.concatenate([
            xq_f,                                       # 98
            aux,                                        # 32
            id98.view(np.float32),                      # 49
            _f8cols(xq_f.astype(F8)),                   # 25
            _f8cols(_pack_w8i(wq, P)),                  # 128
            _f8cols(_pack_w8i(wk, P)),                  # 128
            _f8cols(xkv_f.astype(F8)),                  # 98
            _f8cols(xkvi),                              # 98
            _f8cols(wpcs),                              # 1
        ], axis=1)
        blobB = np.concatenate([
            _f8cols(_pack_w8(wv)),                      # 128
            bv_rep.view(np.float32),                    # 128
            _f8cols(_pack_w8i(wp, P)),                  # 128
        ], axis=1)
        if FFN_BF16:
            blobC = np.concatenate([
                _pack_wbf(w1).view(np.float32),
                _pack_wbf(w2).view(np.float32),
            ], axis=1)
        else:
            blobC = np.concatenate([
                _f8cols(_pack_w8i(w1, P)),                  # 512
                _f8cols(_pack_w8i(w2, P, mc_major=True)),   # 512
            ], axis=1)
        maps.append({
            'blobA': np.ascontiguousarray(blobA),
            'blobB': np.ascontiguousarray(blobB),
            'blobC': np.ascontiguousarray(blobC),
        })
    return maps


def _recon(x):
    x = x.reshape(14, 14, 16, 16)
    x = np.transpose(x, (2, 3, 0, 1))
    return x.reshape(1, 1, 224, 224)


def assemble(core_outs):
    ov = np.concatenate([core_outs[c].T for c in range(4)], axis=0)
    oi = np.concatenate([core_outs[c].T for c in range(4, 8)], axis=0)
    return np.concatenate([_recon(oi), _recon(ov)], axis=1).astype(np.float32)


def get_nc():
    if 'nc' not in _CACHE:
        _CACHE['nc'] = build_bass()
    return _CACHE['nc']


def kernel(**inputs):
    from concourse import bass_utils
    nc = get_nc()
    in_maps = make_in_maps(inputs)
    res = bass_utils.run_bass_kernel_spmd(nc, in_maps, core_ids=list(range(NCORES)))
    outs = [np.asarray(r['out'], np.float32) for r in res.results]
    return assemble(outs)


# revision 65
# speedup vs baseline: 1.2230x; 1.0325x over previous
"""CrossViT fused block on 8 TRN2 NeuronCores.

Sharding: 2 branches (vis-output / ir-output) x 4-way token split -> 8 cores,
no collectives. Each core computes 49 output tokens of one branch end-to-end:
LN1, cross-attention (its queries vs all 196 keys/values of the other
modality), projection, residual, LN2, FFN, residual. Activations are kept
feature-major (features on SBUF partitions) so every linear layer is
matmul(lhsT=W_natural, rhs=xT).

Engine plan: no GpSimd (its microcode library swaps cost ~5us each).
Weight matmuls (q/k/v/proj/FFN) run in fp8e4 DoubleRow perf mode: both
128-row K-tiles of the K=256 contraction in one PE instruction. Weights are
scaled x32 on the host to dodge e4m3 denormals; the 1/32 rides existing
bias/scale slots (tensor_scalar, activation scale). Scores/attnV/transposes
stay bf16. Softmax normalization is a DVE tensor_scalar multiply by the
reciprocal exp-sum; LayerNorm mean/rstd broadcasts are tiny PE outer
products against a ones row; LN2's mean comes from colsum(Wp) @ oT before
the residual lands. ACT does only Exp/Sqrt/Gelu plus table-free
Identity/Square offloads.
"""
import sys
if '/opt/trn_rl_repo' not in sys.path:
    sys.path.insert(0, '/opt/trn_rl_repo')

import numpy as np
import ml_dtypes

BF = ml_dtypes.bfloat16
F8 = ml_dtypes.float8_e4m3fn
N, EMB, H, DH, HID = 196, 256, 8, 32, 1024
T = 49            # tokens per core
EPS, SCALE = 1e-5, 16.0
WS = 32.0         # host-side fp8 weight scale
P = 128
NCORES = 8
TOKC = ((0, 128), (128, 68))   # token chunks of the 196 keys/values

# blobA f32 cols: [0:98] xq f32 | [98:130] aux | [130:179] id98(bf16) |
#   [179:204] xq(f8) | [204:332] wq(f8i) | [332:460] wk(f8i) |
#   [460:558] xkv(f8, k rhs) | [558:686] xkv(f8i, v lhsT, 2x128 tokens) |
#   [686:687] wpcs(f8)
CA = 687
# blobB: [0:128] wv(f8) | [128:256] bv replicated(bf16) | [256:384] wp(f8i)
CB = 384
# blobC: [0:512] w1(f8i) | [512:1024] w2(f8i)  (bf16 fallback: 2048 cols)
import os
FFN_BF16 = bool(int(os.environ.get('FFN_BF16', '0')))
CC = 2048 if FFN_BF16 else 1024

_CACHE = {}


# ---------------------------------------------------------------- bass build
def build_bass():
    import concourse.bacc as bacc
    import concourse.mybir as mybir
    import concourse.tile as tile

    f32 = mybir.dt.float32
    bf16 = mybir.dt.bfloat16
    f8 = mybir.dt.float8e4
    AF = mybir.ActivationFunctionType
    OP = mybir.AluOpType
    DR = mybir.MatmulPerfMode.DoubleRowSwInterleave

    nc = bacc.Bacc("TRN2", target_bir_lowering=False)

    bA_d = nc.dram_tensor("blobA", [P, CA], f32, kind="ExternalInput")
    bB_d = nc.dram_tensor("blobB", [P, CB], f32, kind="ExternalInput")
    bC_d = nc.dram_tensor("blobC", [P, CC], f32, kind="ExternalInput")
    out_d = nc.dram_tensor("out", [EMB, T], f32, kind="ExternalOutput")

    with tile.TileContext(nc) as tc:
        with (
            tc.tile_pool(name="const", bufs=1) as cpool,
            tc.tile_pool(name="act", bufs=1) as apool,
            tc.tile_pool(name="attp", bufs=2) as attpool,
            tc.tile_pool(name="ps_mm", bufs=2, space="PSUM") as ps_mm,
            tc.tile_pool(name="ps_s", bufs=2, space="PSUM") as ps_s,
            tc.tile_pool(name="ps_t", bufs=2, space="PSUM") as ps_t,
            tc.tile_pool(name="ps_big", bufs=1, space="PSUM") as ps_big,
            tc.tile_pool(name="ps_ln", bufs=1, space="PSUM") as ps_ln,
        ):
            # ---- constants / scratch (no input deps; runs during DMA wait)
            ones_row = cpool.tile([1, P], bf16, tag="ones_row")
            nc.vector.memset(ones_row[:], 1.0)
            ones_col = cpool.tile([P, 1], bf16, tag="ones_col")
            nc.vector.memset(ones_col[:], 1.0)
            epst = cpool.tile([1, 1], f32, tag="epst")
            nc.vector.memset(epst[:], EPS)
            # block-diag q tiles: bdt[0] holds pairs 0,1 / bdt[1] pairs 2,3
            bdt = []
            for i in range(2):
                bd = apool.tile([P, 2 * T], bf16, tag=f"bd{i}")
                nc.vector.memset(bd[:], 0.0)
                bdt.append(bd)

            # ---- input DMAs, ordered by first use
            bA = cpool.tile([P, CA], f32, tag="bA")
            nc.sync.dma_start(bA[:], bA_d[:, :])
            bB = cpool.tile([P, CB], f32, tag="bB")
            nc.sync.dma_start(bB[:], bB_d[:, :])
            bC = cpool.tile([P, CC], f32, tag="bC")
            nc.sync.dma_start(bC[:], bC_d[:, :])

            # ---- views
            xq3 = bA[:, 0:98].rearrange("p (c t) -> p c t", c=2)
            aux = bA[:, 98:130]
            id98 = bA[:, 130:179].bitcast(bf16)[0:98, 0:98]
            xq8 = bA[:, 179:204].bitcast(f8)[:, 0:98].rearrange(
                "p (c t) -> p c t", c=2)
            wq = bA[:, 204:332].bitcast(f8).rearrange("p (c m) -> p c m", c=2)
            wk = bA[:, 332:460].bitcast(f8).rearrange("p (c m) -> p c m", c=2)
            xkv = bA[:, 460:558].bitcast(f8).rearrange("p (c t) -> p c t", c=2)
            xkvi = bA[:, 558:686].bitcast(f8)     # [128, 512] interleaved/padded
            wpcs = bA[:, 686:687].bitcast(f8)   # [128, 4]: kc0@0, kc1@2
            bq_c = aux[:, 0:2]
            bk_c = aux[:, 2:4]
            b2_c = aux[:, 6:8]
            b1_c = aux[:, 8:16]
            sumbp = aux[0:1, 16:17]
            ln1bp_c = aux[:, 17:19]    # ln1b + bp (for rv via nv')
            ln2b_c = aux[:, 19:21]
            ln1w_c = aux[:, 21:23]
            ln2w_c = aux[:, 23:25]
            ln2bb2_c = aux[:, 25:27]   # ln2b + b2 (for residual via lvb2)
            wv = bB[:, 0:128].bitcast(f8).rearrange("p (c m) -> p c m", c=2)
            bvv = bB[:, 128:256].bitcast(bf16)
            wp = bB[:, 256:CB].bitcast(f8).rearrange("p (b w) -> p b w", b=2)
            if FFN_BF16:
                w1 = bC[:, 0:1024].bitcast(bf16).rearrange(
                    "p (c m) -> p c m", c=2)
                w2 = bC[:, 1024:CC].bitcast(bf16).rearrange(
                    "p (c m) -> p c m", c=8)
            else:
                w1 = bC[:, 0:512].bitcast(f8).rearrange(
                    "p (b w) -> p b w", b=8)
                w2 = bC[:, 512:CC].bitcast(f8).rearrange(
                    "p (b i w) -> p b i w", b=2, i=4)

            xq_bf = apool.tile([P, 2, T], bf16, tag="xq_bf")
            nc.vector.tensor_copy(xq_bf[:], xq3)

            # packed LN PSUM bank: col views
            ln = ps_ln.tile([P, 512], f32, tag="ln")
            lnp1 = ln[0:1, 0:98].rearrange("p (a t) -> p a t", a=2)
            msum = ln[0:1, 98:147]
            ssq2 = ln[0:1, 147:196]
            Mb1 = ln[:, 196:245]
            Mb2 = ln[:, 245:294]
            R1 = ln[:, 294:343]
            R2 = ln[:, 343:392]

            # ---------------- q dense fp8 DoubleRow (1 matmul/chunk),
            # lane-aligned scatter into block-diag pair tiles (undoes the x32)
            wq_i = wq.rearrange("p c m -> p (c m)").rearrange(
                "p (b w) -> p b w", b=2)
            for mc in range(2):
                qp = ps_mm.tile([P, 512], f32, tag="mm")
                nc.tensor.matmul(qp[:, 0:T], wq_i[:, mc], xq8[:],
                                 start=True, stop=True, perf_mode=DR)
                for hh in range(4):
                    j = hh % 2           # position within pair
                    r0 = hh * DH
                    nc.vector.tensor_scalar(
                        bdt[mc][r0:r0 + DH, j * T:(j + 1) * T],
                        qp[r0:r0 + DH, 0:T], 1.0 / WS,
                        bq_c[r0:r0 + DH, mc:mc + 1],
                        op0=OP.mult, op1=OP.add)

            # ---------------- k pair-merged fp8 DoubleRow: [128, 196]/half
            # pk tiles borrow the big/attT PSUM slots (their later users start
            # well after the k bias reads) so k doesn't WAR-wait on q's slots
            wk_i = wk.rearrange("p c m -> p (c m)").rearrange(
                "p (b w) -> p b w", b=2)
            kt = []
            for i in range(2):
                pool = ps_big if i == 0 else ps_t
                pk = pool.tile([P, 512], f32, tag="big" if i == 0 else "attT")
                nc.tensor.matmul(pk[:, 0:N], wk_i[:, i], xkv[:],
                                 start=True, stop=True, perf_mode=DR)
                k = apool.tile([P, N], bf16, tag=f"k{i}")
                nc.scalar.activation(k[:], pk[:, 0:N], AF.Identity,
                                     scale=1.0 / WS, bias=bk_c[:, i:i + 1])
                kt.append(k)

            # ---------------- v token-major fp8 DoubleRow
            v_bf = []
            for tcx, (t0, tsz) in enumerate(TOKC):
                pv = ps_mm.tile([P, 512], f32, tag="mm")
                nc.tensor.matmul(pv[:, 0:EMB],
                                 xkvi[:, 256 * tcx:256 * (tcx + 1)],
                                 wv[:], start=True, stop=True, perf_mode=DR)
                vt = apool.tile([P, EMB], bf16, tag=f"v{tcx}")
                nc.vector.scalar_tensor_tensor(vt[:tsz], pv[:tsz, 0:EMB],
                                               1.0 / WS, bvv[0:tsz],
                                               op0=OP.mult, op1=OP.add)
                v_bf.append(vt)

            # ---------------- LN1 stats (off critical path)
            xsq1 = apool.tile([P, 2, 2, T], bf16, tag="xsq1")
            nc.vector.tensor_copy(xsq1[:, :, 0], xq_bf[:])
            nc.vector.tensor_tensor(xsq1[:, :, 1], xq_bf[:], xq_bf[:], op=OP.mult)
            for kc in range(2):
                nc.tensor.matmul(lnp1.rearrange("p a t -> p (a t)"), ones_col[:],
                                 xsq1[:, kc].rearrange("p a t -> p (a t)"),
                                 start=(kc == 0), stop=(kc == 1))

            # LN1 mean/rstd chain immediately (so its Sqrt is ready before the
            # first Exp and the ACT queue stays grouped: Sqrt, then all Exps)
            m1 = apool.tile([1, T], f32, tag="m1")
            nc.vector.tensor_scalar(m1[:], lnp1[0:1, 0], 1.0 / EMB, None,
                                    op0=OP.mult)
            m2x1 = apool.tile([1, T], f32, tag="m2x1")
            nc.vector.tensor_tensor(m2x1[:], m1[:], m1[:], op=OP.mult)
            d1 = apool.tile([1, T], f32, tag="d1")
            nc.vector.scalar_tensor_tensor(d1[:], lnp1[0:1, 1], 1.0 / EMB,
                                           m2x1[:], op0=OP.mult, op1=OP.subtract)
            vstd1 = apool.tile([1, T], f32, tag="vstd1")
            nc.scalar.activation(vstd1[:], d1[:], AF.Sqrt, bias=epst[0:1, 0:1])
            r1 = apool.tile([1, T], f32, tag="r1")
            nc.vector.reciprocal_approx_fast(r1[:], vstd1[:])
            mr1_bf = apool.tile([1, 2, T], bf16, tag="mr1_bf")
            nc.vector.tensor_copy(mr1_bf[0:1, 0], m1[:])
            nc.vector.tensor_copy(mr1_bf[0:1, 1], r1[:])

            # ---------------- scores + softmax (recip + scale on DVE)
            ssum = apool.tile([2 * T, 4], f32, tag="ssum")
            rsum = apool.tile([2 * T, 4], f32, tag="rsum")
            att_tiles = []
            for pr in range(4):
                prow = (pr % 2) * 64
                pss = ps_s.tile([2 * T, 512], f32, tag="scores")
                nc.tensor.matmul(pss[:, 0:N], bdt[pr // 2][prow:prow + 64, :],
                                 kt[pr // 2][prow:prow + 64, :],
                                 start=True, stop=True)
                atf = attpool.tile([2 * T, N], bf16, tag="attf")
                nc.scalar.activation(atf[:], pss[:, 0:N], AF.Exp, scale=1.0 / SCALE,
                                     accum_out=ssum[:, pr:pr + 1])
                nc.vector.reciprocal(rsum[:, pr:pr + 1], ssum[:, pr:pr + 1])
                att = attpool.tile([2 * T, N], bf16, tag="attn")
                nc.vector.tensor_scalar(att[:], atf[:], rsum[:, pr:pr + 1],
                                        None, op0=OP.mult)
                att_tiles.append(att)

            # ---------------- attT via PE transpose; pair-merged attnV
            oT8 = apool.tile([P, 2, T], f8, tag="oT8")
            sc = ps_big.tile([P, 512], f32, tag="big")
            for pr in range(4):
                prow = (pr % 2) * 64
                c0 = (pr // 2) * 2 * T
                ats = []
                for tcx, (t0, tsz) in enumerate(TOKC):
                    pt = ps_t.tile([P, 1024], bf16, tag="attT")
                    nc.tensor.transpose(pt[:tsz, 0:2 * T],
                                        att_tiles[pr][:, t0:t0 + tsz], id98[:])
                    at = attpool.tile([P, 2 * T], bf16, tag="atT")
                    nc.vector.tensor_copy(at[:tsz], pt[:tsz, 0:2 * T])
                    ats.append(at)
                for tcx, (t0, tsz) in enumerate(TOKC):
                    nc.tensor.matmul(sc[prow:prow + 64, c0:c0 + 2 * T],
                                     v_bf[tcx][:tsz, pr * 64:(pr + 1) * 64],
                                     ats[tcx][:tsz], start=(tcx == 0),
                                     stop=(tcx == 1), tile_position=(0, prow))
                nc.vector.tensor_copy(oT8[prow:prow + DH, pr // 2],
                                      sc[prow:prow + DH, c0:c0 + T])
                nc.vector.tensor_copy(oT8[prow + DH:prow + 64, pr // 2],
                                      sc[prow + DH:prow + 64, c0 + T:c0 + 2 * T])

            # ---------------- LN1 finish: mean/rstd + outer-product broadcast
            nc.tensor.matmul(Mb1, ones_row[:], mr1_bf[0:1, 0], start=True,
                             stop=True)
            nc.tensor.matmul(R1, ones_row[:], mr1_bf[0:1, 1], start=True,
                             stop=True)
            t1a = apool.tile([P, 2, T], f32, tag="t1a")
            for kc in range(2):
                nc.vector.tensor_tensor(t1a[:, kc], xq3[:, kc], Mb1,
                                        op=OP.subtract)
            u1 = apool.tile([P, 2, T], f32, tag="u1")
            for kc in range(2):
                nc.vector.scalar_tensor_tensor(u1[:, kc], t1a[:, kc],
                                               ln1w_c[:, kc:kc + 1], R1,
                                               op0=OP.mult, op1=OP.mult)
            # nv' = nv + bp (bp folded in so rv = pp/32 + nv')
            nv_bf = apool.tile([P, 2, T], bf16, tag="nv_bf")
            for kc in range(2):
                nc.scalar.add(nv_bf[:, kc], u1[:, kc], ln1bp_c[:, kc:kc + 1])

            # ---------------- LN2 mean, early: sum_f rv = sum_f nv'
            #                  + colsum(Wp)@oT/32  (before rv exists)
            nc.tensor.matmul(msum, ones_col[:], nv_bf[:, 0], start=True,
                             stop=False)
            nc.tensor.matmul(msum, ones_col[:], nv_bf[:, 1], start=False,
                             stop=False)
            for kc in range(2):
                nc.tensor.matmul(msum, wpcs[:, 2 * kc:2 * kc + 1], oT8[:, kc],
                                 start=False, stop=(kc == 1))
            m2 = apool.tile([1, T], f32, tag="m2")
            nc.vector.tensor_scalar(m2[:], msum, 1.0 / EMB, None, op0=OP.mult)
            m2x2 = apool.tile([1, T], f32, tag="m2x2")
            nc.vector.tensor_tensor(m2x2[:], m2[:], m2[:], op=OP.mult)
            m2_bf = apool.tile([1, T], bf16, tag="m2_bf")
            nc.vector.tensor_copy(m2_bf[:], m2[:])
            nc.tensor.matmul(Mb2, ones_row[:], m2_bf[:], start=True, stop=True)

            # ---------------- projection fp8 DoubleRow + residual
            pp = ps_mm.tile([P, 512], f32, tag="mm")
            for mc in range(2):
                # DR psum dst needs >=8B column alignment -> 64-col slots
                nc.tensor.matmul(pp[:, mc * 64:mc * 64 + T], wp[:, mc],
                                 oT8[:], start=True, stop=True, perf_mode=DR)
            rv = apool.tile([P, 2, T], f32, tag="rv")
            for mc in range(2):
                nc.vector.scalar_tensor_tensor(rv[:, mc], pp[:, mc * 64:mc * 64 + T],
                                               1.0 / WS, nv_bf[:, mc],
                                               op0=OP.mult, op1=OP.add)

            # ---------------- LN2: stats after rv; apply via outer products
            t2 = apool.tile([P, 2, T], f32, tag="t2")
            for kc in range(2):
                nc.vector.tensor_tensor(t2[:, kc], rv[:, kc], Mb2,
                                        op=OP.subtract)
            sq2 = apool.tile([P, 2, T], bf16, tag="sq2")
            nc.scalar.square(sq2[:], rv[:])
            for kc in range(2):
                nc.tensor.matmul(ssq2, ones_col[:], sq2[:, kc],
                                 start=(kc == 0), stop=(kc == 1))
            d2 = apool.tile([1, T], f32, tag="d2")
            nc.vector.scalar_tensor_tensor(d2[:], ssq2, 1.0 / EMB, m2x2[:],
                                           op0=OP.mult, op1=OP.subtract)
            vstd2 = apool.tile([1, T], f32, tag="vstd2")
            nc.scalar.activation(vstd2[:], d2[:], AF.Sqrt, bias=epst[0:1, 0:1])
            scrg = apool.tile([1, 1], f32, tag="scrg")
            gfd = AF.Identity if _CACHE.get('sim_ident_gelu') else AF.Gelu
            nc.scalar.activation(scrg[:], vstd2[0:1, 0:1], gfd)
            r2 = apool.tile([1, T], f32, tag="r2")
            nc.vector.reciprocal_approx_fast(r2[:], vstd2[:])
            r2_bf = apool.tile([1, T], bf16, tag="r2_bf")
            nc.vector.tensor_copy(r2_bf[:], r2[:])
            nc.tensor.matmul(R2, ones_row[:], r2_bf[:], start=True, stop=True)
            u2 = apool.tile([P, 2, T], f32, tag="u2")
            for kc in range(2):
                nc.vector.scalar_tensor_tensor(u2[:, kc], t2[:, kc],
                                               ln2w_c[:, kc:kc + 1], R2,
                                               op0=OP.mult, op1=OP.mult)
            # lv for the FFN (fp8, +ln2b) on DVE; lv+b2 for the residual on ACT
            lv8 = apool.tile([P, 2, T], bf16 if FFN_BF16 else f8, tag="lv8")
            for kc in range(2):
                nc.vector.tensor_scalar(lv8[:, kc], u2[:, kc],
                                        ln2b_c[:, kc:kc + 1], None, op0=OP.add)
            lvb2 = apool.tile([P, 2, T], bf16, tag="lvb2")
            for kc in range(2):
                nc.scalar.add(lvb2[:, kc], u2[:, kc], ln2bb2_c[:, kc:kc + 1])

            # ---------------- FFN fp8 DoubleRow + residual
            ph = ps_big.tile([P, 512], f32, tag="big")
            g8 = apool.tile([P, 8, T], bf16 if FFN_BF16 else f8, tag="gelu")
            gf = AF.Identity if _CACHE.get('sim_ident_gelu') else AF.Gelu
            for mc in range(8):
                if FFN_BF16:
                    for kc in range(2):
                        nc.tensor.matmul(ph[:, mc * T:(mc + 1) * T],
                                         w1[:, kc, mc * P:(mc + 1) * P],
                                         lv8[:, kc], start=(kc == 0),
                                         stop=(kc == 1))
                    nc.scalar.activation(g8[:, mc], ph[:, mc * T:(mc + 1) * T],
                                         gf, bias=b1_c[:, mc:mc + 1])
                else:
                    nc.tensor.matmul(ph[:, mc * 64:mc * 64 + T], w1[:, mc],
                                     lv8[:], start=True, stop=True, perf_mode=DR)
                    nc.scalar.activation(g8[:, mc], ph[:, mc * 64:mc * 64 + T],
                                         gf, scale=1.0 / WS,
                                         bias=b1_c[:, mc:mc + 1])

            out_sb = apool.tile([P, 2, T], f32, tag="out")
            out_r = out_d.rearrange("(c p) t -> p c t", p=P)
            for mc in range(2):
                pf = ps_mm.tile([P, 512], f32, tag="mm")
                if FFN_BF16:
                    for kc in range(8):
                        nc.tensor.matmul(pf[:, 0:T],
                                         w2[:, kc, mc * P:(mc + 1) * P],
                                         g8[:, kc], start=(kc == 0),
                                         stop=(kc == 7))
                    nc.vector.scalar_tensor_tensor(out_sb[:, mc], pf[:, 0:T],
                                                   1.0, lvb2[:, mc],
                                                   op0=OP.mult, op1=OP.add)
                else:
                    for i in range(4):
                        nc.tensor.matmul(pf[:, 0:T], w2[:, mc, i],
                                         g8[:, 2 * i:2 * i + 2], start=(i == 0),
                                         stop=(i == 3), perf_mode=DR)
                    nc.vector.scalar_tensor_tensor(out_sb[:, mc], pf[:, 0:T],
                                                   1.0 / WS, lvb2[:, mc],
                                                   op0=OP.mult, op1=OP.add)
                if mc == 0:
                    nc.sync.dma_start(out_r[:, mc], out_sb[:, mc])
                else:
                    nc.gpsimd.dma_start(out_r[:, mc], out_sb[:, mc])

    nc.compile()
    return nc


# ---------------------------------------------------------------- host side
def _reorder_qkv(W, b):
    W4 = np.asarray(W, np.float32).reshape(EMB, H, DH, 3)
    b4 = np.asarray(b, np.float32).reshape(H, DH, 3)
    return ([np.ascontiguousarray(W4[:, :, :, i].reshape(EMB, EMB)) for i in range(3)],
            [np.ascontiguousarray(b4[:, :, i].reshape(EMB)) for i in range(3)])


def _pack_wbf(w):
    w = np.asarray(w, np.float32)
    k, m = w.shape
    c = k // P
    return np.transpose(w.reshape(c, P, m), (1, 0, 2)).reshape(P, c * m).astype(BF)


def _pack_w8(w):
    """(K, M) f32 -> x32-scaled partition-major (128, K//128 * M) f8 block."""
    w = np.asarray(w, np.float32) * WS
    k, m = w.shape
    c = k // P
    return np.transpose(w.reshape(c, P, m), (1, 0, 2)).reshape(P, c * m).astype(F8)


def _ileave(pair3):
    """[P, 2, m] -> [P, 2m]: A/B column-interleaved, columns reversed
    (DoubleRowSwInterleave weight layout)."""
    blk = pair3[:, :, ::-1]
    return np.ascontiguousarray(np.transpose(blk, (0, 2, 1)).reshape(P, -1))


def _pack_w8i(w, mc_cols, mc_major=False):
    """(K, M) f32 -> x32-scaled SwInterleave layout: one [P, 2*mc_cols]
    interleaved block per (k-tile pair, M-chunk)."""
    w = np.asarray(w, np.float32) * WS
    k, m = w.shape
    c = k // P
    w3 = np.transpose(w.reshape(c, P, m), (1, 0, 2)).astype(F8)  # [P, c, m]
    blk = [[_ileave(w3[:, 2 * i:2 * i + 2, j0:j0 + mc_cols])
            for j0 in range(0, m, mc_cols)] for i in range(c // 2)]
    if mc_major:
        out = [blk[i][j] for j in range(len(blk[0])) for i in range(len(blk))]
    else:
        out = [blk[i][j] for i in range(len(blk)) for j in range(len(blk[0]))]
    return np.concatenate(out, axis=1)


def _pack_x(x):
    """(tokens, 256) -> (128, 2*tokens) f32 partition-major transposed."""
    xt = np.ascontiguousarray(np.asarray(x, np.float32).T)       # (256, t)
    t = xt.shape[1]
    return np.transpose(xt.reshape(2, P, t), (1, 0, 2)).reshape(P, 2 * t)


def _f8cols(a):
    """(128, n) f8 array -> zero-padded f32-col view (n_f32 = ceil(n/4))."""
    n = a.shape[1]
    pad = (-n) % 4
    if pad:
        a = np.concatenate([a, np.zeros((P, pad), F8)], axis=1)
    return np.ascontiguousarray(a).view(np.float32)


def _cols(v):
    """(256,) -> (128, 2) natural feature chunks."""
    return np.ascontiguousarray(np.asarray(v, np.float32).reshape(2, P).T)


def make_in_maps(inputs):
    inp = {k: np.asarray(v, np.float32) for k, v in inputs.items()}
    qkv_v = _reorder_qkv(inp['Wqkv_v'], inp['bqkv_v'])
    qkv_i = _reorder_qkv(inp['Wqkv_i'], inp['bqkv_i'])
    maps = []
    for core in range(NCORES):
        branch = core // 4
        r0 = (core % 4) * T
        if branch == 0:   # vis output: vis queries, ir keys/values
            x_own, x_oth = inp['vis_emb'][0], inp['ir_emb'][0]
            wq, bq = qkv_v[0][0], qkv_v[1][0]
            wk, bk = qkv_i[0][1], qkv_i[1][1]
            wv, bv = qkv_i[0][2], qkv_i[1][2]
            wp, bp = inp['Wp_v'], inp['bp_v']
            lnp = (inp['ln1v_w'], inp['ln1v_b'], inp['ln2v_w'], inp['ln2v_b'])
            w1, b1, w2, b2 = inp['W1v'], inp['b1v'], inp['W2v'], inp['b2v']
        else:             # ir output: ir queries, vis kv
            x_own, x_oth = inp['ir_emb'][0], inp['vis_emb'][0]
            wq, bq = qkv_i[0][0], qkv_i[1][0]
            wk, bk = qkv_v[0][1], qkv_v[1][1]
            wv, bv = qkv_v[0][2], qkv_v[1][2]
            wp, bp = inp['Wp_i'], inp['bp_i']
            lnp = (inp['ln1i_w'], inp['ln1i_b'], inp['ln2i_w'], inp['ln2i_b'])
            w1, b1, w2, b2 = inp['W1i'], inp['b1i'], inp['W2i'], inp['b2i']

        aux = np.zeros((P, 32), np.float32)
        aux[:, 0:2] = _cols(bq)
        aux[:, 2:4] = _cols(bk)
        aux[:, 4:6] = _cols(bp)
        aux[:, 6:8] = _cols(b2)
        aux[:, 8:16] = np.asarray(b1, np.float32).reshape(8, P).T
        aux[0, 16] = float(np.sum(np.asarray(bp, np.float32))) / EMB
        aux[:, 17:19] = _cols(np.asarray(lnp[1], np.float32)
                              + np.asarray(bp, np.float32))
        aux[:, 19:21] = _cols(lnp[3])
        aux[:, 21:23] = _cols(lnp[0])
        aux[:, 23:25] = _cols(lnp[2])
        aux[:, 25:27] = _cols(np.asarray(lnp[3], np.float32)
                              + np.asarray(b2, np.float32))

        id98 = np.zeros((P, 98), BF)
        id98[0:98, 0:98] = np.eye(98, dtype=BF)

        xq_f = _pack_x(x_own[r0:r0 + T])
        xkv_f = _pack_x(x_oth)
        xkv3 = np.zeros((P, 2, 256), F8)
        xkv3[:, :, 0:N] = xkv_f.astype(F8).reshape(P, 2, N)
        xkvi = np.concatenate(
            [_ileave(xkv3[:, :, 0:128]), _ileave(xkv3[:, :, 128:256])], axis=1)
        wpcs = np.zeros((P, 4), F8)
        wpcs[:, 0:4:2] = np.asarray(
            wp, np.float32).sum(axis=1).reshape(2, P).T.astype(F8)

        bv_rep = np.ascontiguousarray(np.broadcast_to(
            np.asarray(bv, np.float32).astype(BF)[None, :], (P, EMB)))

        blobA = np.concatenate([
            xq_f,                                       # 98
            aux,                                        # 32
            id98.view(np.float32),                      # 49
            _f8cols(xq_f.astype(F8)),                   # 25
            _f8cols(_pack_w8i(wq, P)),                  # 128
            _f8cols(_pack_w8i(wk, P)),                  # 128
            _f8cols(xkv_f.astype(F8)),                  # 98
            _f8cols(xkvi),                              # 98
            _f8cols(wpcs),                              # 1
        ], axis=1)
        blobB = np.concatenate([
            _f8cols(_pack_w8(wv)),                      # 128
            bv_rep.view(np.float32),                    # 128
            _f8cols(_pack_w8i(wp, P)),                  # 128
        ], axis=1)
        if FFN_BF16:
            blobC = np.concatenate([
                _pack_wbf(w1).view(np.float32),
                _pack_wbf(w2).view(np.float32),
            ], axis=1)
        else:
            blobC = np.concatenate([
                _f8cols(_pack_w8i(w1, P)),                  # 512
                _f8cols(_pack_w8i(w2, P, mc_major=True)),   # 512
            ], axis=1)
        maps.append({
            'blobA': np.ascontiguousarray(blobA),
            'blobB': np.ascontiguousarray(blobB),
            'blobC': np.ascontiguousarray(blobC),
        })
    return maps


def _recon(x):
    x = x.reshape(14, 14, 16, 16)
    x = np.transpose(x, (2, 3, 0, 1))
    return x.reshape(1, 1, 224, 224)


def assemble(core_outs):
    ov = np.concatenate([core_outs[c].T for c in range(4)], axis=0)
    oi = np.concatenate([core_outs[c].T for c in range(4, 8)], axis=0)
    return np.concatenate([_recon(oi), _recon(ov)], axis=1).astype(np.float32)


def get_nc():
    if 'nc' not in _CACHE:
        _CACHE['nc'] = build_bass()
    return _CACHE['nc']


def kernel(**inputs):
    from concourse import bass_utils
    nc = get_nc()
    in_maps = make_in_maps(inputs)
    res = bass_utils.run_bass_kernel_spmd(nc, in_maps, core_ids=list(range(NCORES)))
    outs = [np.asarray(r['out'], np.float32) for r in res.results]
    return assemble(outs)


# revision 66
# speedup vs baseline: 1.2407x; 1.0145x over previous
"""CrossViT fused block on 8 TRN2 NeuronCores.

Sharding: 2 branches (vis-output / ir-output) x 4-way token split -> 8 cores,
no collectives. Each core computes 49 output tokens of one branch end-to-end:
LN1, cross-attention (its queries vs all 196 keys/values of the other
modality), projection, residual, LN2, FFN, residual. Activations are kept
feature-major (features on SBUF partitions) so every linear layer is
matmul(lhsT=W_natural, rhs=xT).

Engine plan: no GpSimd (its microcode library swaps cost ~5us each).
Weight matmuls (q/k/v/proj/FFN) run in fp8e4 DoubleRow perf mode: both
128-row K-tiles of the K=256 contraction in one PE instruction. Weights are
scaled x32 on the host to dodge e4m3 denormals; the 1/32 rides existing
bias/scale slots (tensor_scalar, activation scale). Scores/attnV/transposes
stay bf16. Softmax normalization is a DVE tensor_scalar multiply by the
reciprocal exp-sum; LayerNorm mean/rstd broadcasts are tiny PE outer
products against a ones row; LN2's mean comes from colsum(Wp) @ oT before
the residual lands. ACT does only Exp/Sqrt/Gelu plus table-free
Identity/Square offloads.
"""
import sys
if '/opt/trn_rl_repo' not in sys.path:
    sys.path.insert(0, '/opt/trn_rl_repo')

import numpy as np
import ml_dtypes

BF = ml_dtypes.bfloat16
F8 = ml_dtypes.float8_e4m3fn
N, EMB, H, DH, HID = 196, 256, 8, 32, 1024
T = 49            # tokens per core
EPS, SCALE = 1e-5, 16.0
WS = 32.0         # host-side fp8 weight scale
P = 128
NCORES = 8
TOKC = ((0, 128), (128, 68))   # token chunks of the 196 keys/values

# blobA f32 cols: [0:98] xq f32 | [98:130] aux | [130:179] id98(bf16) |
#   [179:204] xq(f8) | [204:332] wq(f8i) | [332:460] wk(f8i) |
#   [460:558] xkv(f8, k rhs) | [558:686] xkv(f8i, v lhsT, 2x128 tokens) |
#   [686:687] wpcs(f8)
CA = 687
# blobB: [0:128] wv(f8) | [128:256] bv replicated(bf16) | [256:384] wp(f8i)
CB = 384
# blobC: [0:512] w1(f8i) | [512:1024] w2(f8i)  (bf16 fallback: 2048 cols)
import os
FFN_BF16 = bool(int(os.environ.get('FFN_BF16', '0')))
CC = 2048 if FFN_BF16 else 1024

_CACHE = {}


# ---------------------------------------------------------------- bass build
def build_bass():
    import concourse.bacc as bacc
    import concourse.mybir as mybir
    import concourse.tile as tile

    f32 = mybir.dt.float32
    bf16 = mybir.dt.bfloat16
    f8 = mybir.dt.float8e4
    AF = mybir.ActivationFunctionType
    OP = mybir.AluOpType
    DR = mybir.MatmulPerfMode.DoubleRowSwInterleave

    nc = bacc.Bacc("TRN2", target_bir_lowering=False)

    bA_d = nc.dram_tensor("blobA", [P, CA], f32, kind="ExternalInput")
    bB_d = nc.dram_tensor("blobB", [P, CB], f32, kind="ExternalInput")
    bC_d = nc.dram_tensor("blobC", [P, CC], f32, kind="ExternalInput")
    out_d = nc.dram_tensor("out", [EMB, T], f32, kind="ExternalOutput")

    with tile.TileContext(nc) as tc:
        with (
            tc.tile_pool(name="const", bufs=1) as cpool,
            tc.tile_pool(name="act", bufs=1) as apool,
            tc.tile_pool(name="attp", bufs=2) as attpool,
            tc.tile_pool(name="ps_mm", bufs=2, space="PSUM") as ps_mm,
            tc.tile_pool(name="ps_s", bufs=2, space="PSUM") as ps_s,
            tc.tile_pool(name="ps_t", bufs=2, space="PSUM") as ps_t,
            tc.tile_pool(name="ps_big", bufs=1, space="PSUM") as ps_big,
            tc.tile_pool(name="ps_ln", bufs=1, space="PSUM") as ps_ln,
        ):
            # ---- constants / scratch (no input deps; runs during DMA wait)
            ones_row = cpool.tile([1, P], bf16, tag="ones_row")
            nc.vector.memset(ones_row[:], 1.0)
            ones_col = cpool.tile([P, 1], bf16, tag="ones_col")
            nc.vector.memset(ones_col[:], 1.0)
            epst = cpool.tile([1, 1], f32, tag="epst")
            nc.vector.memset(epst[:], EPS)
            # block-diag q tiles: bdt[0] holds pairs 0,1 / bdt[1] pairs 2,3
            bdt = []
            for i in range(2):
                bd = apool.tile([P, 2 * T], bf16, tag=f"bd{i}")
                nc.vector.memset(bd[:], 0.0)
                bdt.append(bd)

            # ---- input DMAs, ordered by first use
            bA = cpool.tile([P, CA], f32, tag="bA")
            nc.sync.dma_start(bA[:], bA_d[:, :])
            bB = cpool.tile([P, CB], f32, tag="bB")
            nc.sync.dma_start(bB[:], bB_d[:, :])
            bC = cpool.tile([P, CC], f32, tag="bC")
            nc.sync.dma_start(bC[:], bC_d[:, :])

            # ---- views
            xq3 = bA[:, 0:98].rearrange("p (c t) -> p c t", c=2)
            aux = bA[:, 98:130]
            id98 = bA[:, 130:179].bitcast(bf16)[0:98, 0:98]
            xq8 = bA[:, 179:204].bitcast(f8)[:, 0:98].rearrange(
                "p (c t) -> p c t", c=2)
            wq = bA[:, 204:332].bitcast(f8).rearrange("p (c m) -> p c m", c=2)
            wk = bA[:, 332:460].bitcast(f8).rearrange("p (c m) -> p c m", c=2)
            xkv = bA[:, 460:558].bitcast(f8).rearrange("p (c t) -> p c t", c=2)
            xkvi = bA[:, 558:686].bitcast(f8)     # [128, 512] interleaved/padded
            wpcs = bA[:, 686:687].bitcast(f8)   # [128, 4]: kc0@0, kc1@2
            bq_c = aux[:, 0:2]
            bk_c = aux[:, 2:4]
            b2_c = aux[:, 6:8]
            b1_c = aux[:, 8:16]
            sumbp = aux[0:1, 16:17]
            ln1bp_c = aux[:, 17:19]    # ln1b + bp (for rv via nv')
            ln2b_c = aux[:, 19:21]
            ln1w_c = aux[:, 21:23]
            ln2w_c = aux[:, 23:25]
            ln2bb2_c = aux[:, 25:27]   # ln2b + b2 (for residual via lvb2)
            wv = bB[:, 0:128].bitcast(f8).rearrange("p (c m) -> p c m", c=2)
            bvv = bB[:, 128:256].bitcast(bf16)
            wp = bB[:, 256:CB].bitcast(f8).rearrange("p (b w) -> p b w", b=2)
            if FFN_BF16:
                w1 = bC[:, 0:1024].bitcast(bf16).rearrange(
                    "p (c m) -> p c m", c=2)
                w2 = bC[:, 1024:CC].bitcast(bf16).rearrange(
                    "p (c m) -> p c m", c=8)
            else:
                w1 = bC[:, 0:512].bitcast(f8).rearrange(
                    "p (b w) -> p b w", b=8)
                w2 = bC[:, 512:CC].bitcast(f8).rearrange(
                    "p (b i w) -> p b i w", b=2, i=4)

            xq_bf = apool.tile([P, 2, T], bf16, tag="xq_bf")
            nc.vector.tensor_copy(xq_bf[:], xq3)

            # packed LN PSUM bank: col views
            ln = ps_ln.tile([P, 512], f32, tag="ln")
            lnp1 = ln[0:1, 0:98].rearrange("p (a t) -> p a t", a=2)
            msum = ln[0:1, 98:147]
            ssq2 = ln[0:1, 147:196]
            Mb1 = ln[:, 196:245]
            Mb2 = ln[:, 245:294]
            R1 = ln[:, 294:343]
            R2 = ln[:, 343:392]

            # ---------------- q dense fp8 DoubleRow (1 matmul/chunk),
            # lane-aligned scatter into block-diag pair tiles (undoes the x32)
            wq_i = wq.rearrange("p c m -> p (c m)").rearrange(
                "p (b w) -> p b w", b=2)
            for mc in range(2):
                qp = ps_mm.tile([P, 512], f32, tag="mm")
                nc.tensor.matmul(qp[:, 0:T], wq_i[:, mc], xq8[:],
                                 start=True, stop=True, perf_mode=DR)
                for hh in range(4):
                    j = hh % 2           # position within pair
                    r0 = hh * DH
                    nc.vector.tensor_scalar(
                        bdt[mc][r0:r0 + DH, j * T:(j + 1) * T],
                        qp[r0:r0 + DH, 0:T], 1.0 / WS,
                        bq_c[r0:r0 + DH, mc:mc + 1],
                        op0=OP.mult, op1=OP.add)

            # ---------------- k pair-merged fp8 DoubleRow: [128, 196]/half
            # pk tiles borrow the big/attT PSUM slots (their later users start
            # well after the k bias reads) so k doesn't WAR-wait on q's slots
            wk_i = wk.rearrange("p c m -> p (c m)").rearrange(
                "p (b w) -> p b w", b=2)
            kt = []
            for i in range(2):
                pool = ps_big if i == 0 else ps_t
                pk = pool.tile([P, 512], f32, tag="big" if i == 0 else "attT")
                nc.tensor.matmul(pk[:, 0:N], wk_i[:, i], xkv[:],
                                 start=True, stop=True, perf_mode=DR)
                k = apool.tile([P, N], bf16, tag=f"k{i}")
                nc.scalar.activation(k[:], pk[:, 0:N], AF.Identity,
                                     scale=1.0 / WS, bias=bk_c[:, i:i + 1])
                kt.append(k)

            # ---------------- v token-major fp8 DoubleRow
            v_bf = []
            for tcx, (t0, tsz) in enumerate(TOKC):
                pv = ps_mm.tile([P, 512], f32, tag="mm")
                nc.tensor.matmul(pv[:, 0:EMB],
                                 xkvi[:, 256 * tcx:256 * (tcx + 1)],
                                 wv[:], start=True, stop=True, perf_mode=DR)
                vt = apool.tile([P, EMB], bf16, tag=f"v{tcx}")
                nc.vector.scalar_tensor_tensor(vt[:tsz], pv[:tsz, 0:EMB],
                                               1.0 / WS, bvv[0:tsz],
                                               op0=OP.mult, op1=OP.add)
                v_bf.append(vt)

            # ---------------- LN1 stats (off critical path)
            xsq1 = apool.tile([P, 2, 2, T], bf16, tag="xsq1")
            nc.vector.tensor_copy(xsq1[:, :, 0], xq_bf[:])
            nc.vector.tensor_tensor(xsq1[:, :, 1], xq_bf[:], xq_bf[:], op=OP.mult)
            for kc in range(2):
                nc.tensor.matmul(lnp1.rearrange("p a t -> p (a t)"), ones_col[:],
                                 xsq1[:, kc].rearrange("p a t -> p (a t)"),
                                 start=(kc == 0), stop=(kc == 1))

            ssum = apool.tile([2 * T, 4], f32, tag="ssum")
            rsum = apool.tile([2 * T, 4], f32, tag="rsum")
            # LN1 mean chain now; the variance stt is gated on the last
            # softmax reciprocal so LN1's Sqrt follows all 4 Exps on the ACT
            # queue (one resident table; a sandwiched Sqrt costs 2 reloads)
            m1 = apool.tile([1, T], f32, tag="m1")
            nc.vector.tensor_scalar(m1[:], lnp1[0:1, 0], 1.0 / EMB, None,
                                    op0=OP.mult)
            m2x1 = apool.tile([1, T], f32, tag="m2x1")
            nc.vector.scalar_tensor_tensor(m2x1[:], m1[:], rsum[0:1, 3:4],
                                           m1[:], op0=OP.bypass, op1=OP.mult)
            d1 = apool.tile([1, T], f32, tag="d1")
            nc.vector.scalar_tensor_tensor(d1[:], lnp1[0:1, 1], 1.0 / EMB,
                                           m2x1[:], op0=OP.mult, op1=OP.subtract)
            vstd1 = apool.tile([1, T], f32, tag="vstd1")
            nc.scalar.activation(vstd1[:], d1[:], AF.Sqrt, bias=epst[0:1, 0:1])
            r1 = apool.tile([1, T], f32, tag="r1")
            nc.vector.reciprocal_approx_fast(r1[:], vstd1[:])
            mr1_bf = apool.tile([1, 2, T], bf16, tag="mr1_bf")
            nc.vector.tensor_copy(mr1_bf[0:1, 0], m1[:])
            nc.vector.tensor_copy(mr1_bf[0:1, 1], r1[:])

            # ---------------- scores + softmax (recip + scale on DVE)
            att_tiles = []
            for pr in range(4):
                prow = (pr % 2) * 64
                pss = ps_s.tile([2 * T, 512], f32, tag="scores")
                nc.tensor.matmul(pss[:, 0:N], bdt[pr // 2][prow:prow + 64, :],
                                 kt[pr // 2][prow:prow + 64, :],
                                 start=True, stop=True)
                atf = attpool.tile([2 * T, N], bf16, tag="attf")
                nc.scalar.activation(atf[:], pss[:, 0:N], AF.Exp, scale=1.0 / SCALE,
                                     accum_out=ssum[:, pr:pr + 1])
                nc.vector.reciprocal(rsum[:, pr:pr + 1], ssum[:, pr:pr + 1])
                att = attpool.tile([2 * T, N], bf16, tag="attn")
                nc.vector.tensor_scalar(att[:], atf[:], rsum[:, pr:pr + 1],
                                        None, op0=OP.mult)
                att_tiles.append(att)

            # ---------------- attT via PE transpose; pair-merged attnV
            oT8 = apool.tile([P, 2, T], f8, tag="oT8")
            sc = ps_big.tile([P, 512], f32, tag="big")
            for pr in range(4):
                prow = (pr % 2) * 64
                c0 = (pr // 2) * 2 * T
                ats = []
                for tcx, (t0, tsz) in enumerate(TOKC):
                    pt = ps_t.tile([P, 1024], bf16, tag="attT")
                    nc.tensor.transpose(pt[:tsz, 0:2 * T],
                                        att_tiles[pr][:, t0:t0 + tsz], id98[:])
                    at = attpool.tile([P, 2 * T], bf16, tag="atT")
                    nc.vector.tensor_copy(at[:tsz], pt[:tsz, 0:2 * T])
                    ats.append(at)
                for tcx, (t0, tsz) in enumerate(TOKC):
                    nc.tensor.matmul(sc[prow:prow + 64, c0:c0 + 2 * T],
                                     v_bf[tcx][:tsz, pr * 64:(pr + 1) * 64],
                                     ats[tcx][:tsz], start=(tcx == 0),
                                     stop=(tcx == 1), tile_position=(0, prow))
                nc.vector.tensor_copy(oT8[prow:prow + DH, pr // 2],
                                      sc[prow:prow + DH, c0:c0 + T])
                nc.vector.tensor_copy(oT8[prow + DH:prow + 64, pr // 2],
                                      sc[prow + DH:prow + 64, c0 + T:c0 + 2 * T])

            # ---------------- LN1 finish: mean/rstd + outer-product broadcast
            nc.tensor.matmul(Mb1, ones_row[:], mr1_bf[0:1, 0], start=True,
                             stop=True)
            nc.tensor.matmul(R1, ones_row[:], mr1_bf[0:1, 1], start=True,
                             stop=True)
            t1a = apool.tile([P, 2, T], f32, tag="t1a")
            for kc in range(2):
                nc.vector.tensor_tensor(t1a[:, kc], xq3[:, kc], Mb1,
                                        op=OP.subtract)
            u1 = apool.tile([P, 2, T], f32, tag="u1")
            for kc in range(2):
                nc.vector.scalar_tensor_tensor(u1[:, kc], t1a[:, kc],
                                               ln1w_c[:, kc:kc + 1], R1,
                                               op0=OP.mult, op1=OP.mult)
            # nv' = nv + bp (bp folded in so rv = pp/32 + nv')
            nv_bf = apool.tile([P, 2, T], bf16, tag="nv_bf")
            for kc in range(2):
                nc.scalar.add(nv_bf[:, kc], u1[:, kc], ln1bp_c[:, kc:kc + 1])

            # ---------------- LN2 mean, early: sum_f rv = sum_f nv'
            #                  + colsum(Wp)@oT/32  (before rv exists)
            nc.tensor.matmul(msum, ones_col[:], nv_bf[:, 0], start=True,
                             stop=False)
            nc.tensor.matmul(msum, ones_col[:], nv_bf[:, 1], start=False,
                             stop=False)
            for kc in range(2):
                nc.tensor.matmul(msum, wpcs[:, 2 * kc:2 * kc + 1], oT8[:, kc],
                                 start=False, stop=(kc == 1))
            m2 = apool.tile([1, T], f32, tag="m2")
            nc.vector.tensor_scalar(m2[:], msum, 1.0 / EMB, None, op0=OP.mult)
            m2x2 = apool.tile([1, T], f32, tag="m2x2")
            nc.vector.tensor_tensor(m2x2[:], m2[:], m2[:], op=OP.mult)
            m2_bf = apool.tile([1, T], bf16, tag="m2_bf")
            nc.vector.tensor_copy(m2_bf[:], m2[:])
            nc.tensor.matmul(Mb2, ones_row[:], m2_bf[:], start=True, stop=True)

            # ---------------- projection fp8 DoubleRow + residual
            pp = ps_mm.tile([P, 512], f32, tag="mm")
            for mc in range(2):
                # DR psum dst needs >=8B column alignment -> 64-col slots
                nc.tensor.matmul(pp[:, mc * 64:mc * 64 + T], wp[:, mc],
                                 oT8[:], start=True, stop=True, perf_mode=DR)
            rv = apool.tile([P, 2, T], f32, tag="rv")
            for mc in range(2):
                nc.vector.scalar_tensor_tensor(rv[:, mc], pp[:, mc * 64:mc * 64 + T],
                                               1.0 / WS, nv_bf[:, mc],
                                               op0=OP.mult, op1=OP.add)

            # ---------------- LN2: stats after rv; apply via outer products
            t2 = apool.tile([P, 2, T], f32, tag="t2")
            for kc in range(2):
                nc.vector.tensor_tensor(t2[:, kc], rv[:, kc], Mb2,
                                        op=OP.subtract)
            sq2 = apool.tile([P, 2, T], bf16, tag="sq2")
            nc.scalar.square(sq2[:], rv[:])
            for kc in range(2):
                nc.tensor.matmul(ssq2, ones_col[:], sq2[:, kc],
                                 start=(kc == 0), stop=(kc == 1))
            d2 = apool.tile([1, T], f32, tag="d2")
            nc.vector.scalar_tensor_tensor(d2[:], ssq2, 1.0 / EMB, m2x2[:],
                                           op0=OP.mult, op1=OP.subtract)
            vstd2 = apool.tile([1, T], f32, tag="vstd2")
            nc.scalar.activation(vstd2[:], d2[:], AF.Sqrt, bias=epst[0:1, 0:1])
            scrg = apool.tile([1, 1], f32, tag="scrg")
            gfd = AF.Identity if _CACHE.get('sim_ident_gelu') else AF.Gelu
            nc.scalar.activation(scrg[:], vstd2[0:1, 0:1], gfd)
            r2 = apool.tile([1, T], f32, tag="r2")
            nc.vector.reciprocal_approx_fast(r2[:], vstd2[:])
            r2_bf = apool.tile([1, T], bf16, tag="r2_bf")
            nc.vector.tensor_copy(r2_bf[:], r2[:])
            nc.tensor.matmul(R2, ones_row[:], r2_bf[:], start=True, stop=True)
            u2 = apool.tile([P, 2, T], f32, tag="u2")
            for kc in range(2):
                nc.vector.scalar_tensor_tensor(u2[:, kc], t2[:, kc],
                                               ln2w_c[:, kc:kc + 1], R2,
                                               op0=OP.mult, op1=OP.mult)
            # lv for the FFN (fp8, +ln2b) on DVE; lv+b2 for the residual on ACT
            lv8 = apool.tile([P, 2, T], bf16 if FFN_BF16 else f8, tag="lv8")
            for kc in range(2):
                nc.vector.tensor_scalar(lv8[:, kc], u2[:, kc],
                                        ln2b_c[:, kc:kc + 1], None, op0=OP.add)
            lvb2 = apool.tile([P, 2, T], bf16, tag="lvb2")
            for kc in range(2):
                nc.scalar.add(lvb2[:, kc], u2[:, kc], ln2bb2_c[:, kc:kc + 1])

            # ---------------- FFN fp8 DoubleRow + residual
            ph = ps_big.tile([P, 512], f32, tag="big")
            g8 = apool.tile([P, 8, T], bf16 if FFN_BF16 else f8, tag="gelu")
            gf = AF.Identity if _CACHE.get('sim_ident_gelu') else AF.Gelu
            for mc in range(8):
                if FFN_BF16:
                    for kc in range(2):
                        nc.tensor.matmul(ph[:, mc * T:(mc + 1) * T],
                                         w1[:, kc, mc * P:(mc + 1) * P],
                                         lv8[:, kc], start=(kc == 0),
                                         stop=(kc == 1))
                    nc.scalar.activation(g8[:, mc], ph[:, mc * T:(mc + 1) * T],
                                         gf, bias=b1_c[:, mc:mc + 1])
                else:
                    nc.tensor.matmul(ph[:, mc * 64:mc * 64 + T], w1[:, mc],
                                     lv8[:], start=True, stop=True, perf_mode=DR)
                    nc.scalar.activation(g8[:, mc], ph[:, mc * 64:mc * 64 + T],
                                         gf, scale=1.0 / WS,
                                         bias=b1_c[:, mc:mc + 1])

            out_sb = apool.tile([P, 2, T], f32, tag="out")
            out_r = out_d.rearrange("(c p) t -> p c t", p=P)
            for mc in range(2):
                pf = ps_mm.tile([P, 512], f32, tag="mm")
                if FFN_BF16:
                    for kc in range(8):
                        nc.tensor.matmul(pf[:, 0:T],
                                         w2[:, kc, mc * P:(mc + 1) * P],
                                         g8[:, kc], start=(kc == 0),
                                         stop=(kc == 7))
                    nc.vector.scalar_tensor_tensor(out_sb[:, mc], pf[:, 0:T],
                                                   1.0, lvb2[:, mc],
                                                   op0=OP.mult, op1=OP.add)
                else:
                    for i in range(4):
                        nc.tensor.matmul(pf[:, 0:T], w2[:, mc, i],
                                         g8[:, 2 * i:2 * i + 2], start=(i == 0),
                                         stop=(i == 3), perf_mode=DR)
                    nc.vector.scalar_tensor_tensor(out_sb[:, mc], pf[:, 0:T],
                                                   1.0 / WS, lvb2[:, mc],
                                                   op0=OP.mult, op1=OP.add)
                if mc == 0:
                    nc.sync.dma_start(out_r[:, mc], out_sb[:, mc])
                else:
                    nc.gpsimd.dma_start(out_r[:, mc], out_sb[:, mc])

    nc.compile()
    return nc


# ---------------------------------------------------------------- host side
def _reorder_qkv(W, b):
    W4 = np.asarray(W, np.float32).reshape(EMB, H, DH, 3)
    b4 = np.asarray(b, np.float32).reshape(H, DH, 3)
    return ([np.ascontiguousarray(W4[:, :, :, i].reshape(EMB, EMB)) for i in range(3)],
            [np.ascontiguousarray(b4[:, :, i].reshape(EMB)) for i in range(3)])


def _pack_wbf(w):
    w = np.asarray(w, np.float32)
    k, m = w.shape
    c = k // P
    return np.transpose(w.reshape(c, P, m), (1, 0, 2)).reshape(P, c * m).astype(BF)


def _pack_w8(w):
    """(K, M) f32 -> x32-scaled partition-major (128, K//128 * M) f8 block."""
    w = np.asarray(w, np.float32) * WS
    k, m = w.shape
    c = k // P
    return np.transpose(w.reshape(c, P, m), (1, 0, 2)).reshape(P, c * m).astype(F8)


def _ileave(pair3):
    """[P, 2, m] -> [P, 2m]: A/B column-interleaved, columns reversed
    (DoubleRowSwInterleave weight layout)."""
    blk = pair3[:, :, ::-1]
    return np.ascontiguousarray(np.transpose(blk, (0, 2, 1)).reshape(P, -1))


def _pack_w8i(w, mc_cols, mc_major=False):
    """(K, M) f32 -> x32-scaled SwInterleave layout: one [P, 2*mc_cols]
    interleaved block per (k-tile pair, M-chunk)."""
    w = np.asarray(w, np.float32) * WS
    k, m = w.shape
    c = k // P
    w3 = np.transpose(w.reshape(c, P, m), (1, 0, 2)).astype(F8)  # [P, c, m]
    blk = [[_ileave(w3[:, 2 * i:2 * i + 2, j0:j0 + mc_cols])
            for j0 in range(0, m, mc_cols)] for i in range(c // 2)]
    if mc_major:
        out = [blk[i][j] for j in range(len(blk[0])) for i in range(len(blk))]
    else:
        out = [blk[i][j] for i in range(len(blk)) for j in range(len(blk[0]))]
    return np.concatenate(out, axis=1)


def _pack_x(x):
    """(tokens, 256) -> (128, 2*tokens) f32 partition-major transposed."""
    xt = np.ascontiguousarray(np.asarray(x, np.float32).T)       # (256, t)
    t = xt.shape[1]
    return np.transpose(xt.reshape(2, P, t), (1, 0, 2)).reshape(P, 2 * t)


def _f8cols(a):
    """(128, n) f8 array -> zero-padded f32-col view (n_f32 = ceil(n/4))."""
    n = a.shape[1]
    pad = (-n) % 4
    if pad:
        a = np.concatenate([a, np.zeros((P, pad), F8)], axis=1)
    return np.ascontiguousarray(a).view(np.float32)


def _cols(v):
    """(256,) -> (128, 2) natural feature chunks."""
    return np.ascontiguousarray(np.asarray(v, np.float32).reshape(2, P).T)


def make_in_maps(inputs):
    inp = {k: np.asarray(v, np.float32) for k, v in inputs.items()}
    qkv_v = _reorder_qkv(inp['Wqkv_v'], inp['bqkv_v'])
    qkv_i = _reorder_qkv(inp['Wqkv_i'], inp['bqkv_i'])
    maps = []
    for core in range(NCORES):
        branch = core // 4
        r0 = (core % 4) * T
        if branch == 0:   # vis output: vis queries, ir keys/values
            x_own, x_oth = inp['vis_emb'][0], inp['ir_emb'][0]
            wq, bq = qkv_v[0][0], qkv_v[1][0]
            wk, bk = qkv_i[0][1], qkv_i[1][1]
            wv, bv = qkv_i[0][2], qkv_i[1][2]
            wp, bp = inp['Wp_v'], inp['bp_v']
            lnp = (inp['ln1v_w'], inp['ln1v_b'], inp['ln2v_w'], inp['ln2v_b'])
            w1, b1, w2, b2 = inp['W1v'], inp['b1v'], inp['W2v'], inp['b2v']
        else:             # ir output: ir queries, vis kv
            x_own, x_oth = inp['ir_emb'][0], inp['vis_emb'][0]
            wq, bq = qkv_i[0][0], qkv_i[1][0]
            wk, bk = qkv_v[0][1], qkv_v[1][1]
            wv, bv = qkv_v[0][2], qkv_v[1][2]
            wp, bp = inp['Wp_i'], inp['bp_i']
            lnp = (inp['ln1i_w'], inp['ln1i_b'], inp['ln2i_w'], inp['ln2i_b'])
            w1, b1, w2, b2 = inp['W1i'], inp['b1i'], inp['W2i'], inp['b2i']

        aux = np.zeros((P, 32), np.float32)
        aux[:, 0:2] = _cols(bq)
        aux[:, 2:4] = _cols(bk)
        aux[:, 4:6] = _cols(bp)
        aux[:, 6:8] = _cols(b2)
        aux[:, 8:16] = np.asarray(b1, np.float32).reshape(8, P).T
        aux[0, 16] = float(np.sum(np.asarray(bp, np.float32))) / EMB
        aux[:, 17:19] = _cols(np.asarray(lnp[1], np.float32)
                              + np.asarray(bp, np.float32))
        aux[:, 19:21] = _cols(lnp[3])
        aux[:, 21:23] = _cols(lnp[0])
        aux[:, 23:25] = _cols(lnp[2])
        aux[:, 25:27] = _cols(np.asarray(lnp[3], np.float32)
                              + np.asarray(b2, np.float32))

        id98 = np.zeros((P, 98), BF)
        id98[0:98, 0:98] = np.eye(98, dtype=BF)

        xq_f = _pack_x(x_own[r0:r0 + T])
        xkv_f = _pack_x(x_oth)
        xkv3 = np.zeros((P, 2, 256), F8)
        xkv3[:, :, 0:N] = xkv_f.astype(F8).reshape(P, 2, N)
        xkvi = np.concatenate(
            [_ileave(xkv3[:, :, 0:128]), _ileave(xkv3[:, :, 128:256])], axis=1)
        wpcs = np.zeros((P, 4), F8)
        wpcs[:, 0:4:2] = np.asarray(
            wp, np.float32).sum(axis=1).reshape(2, P).T.astype(F8)

        bv_rep = np.ascontiguousarray(np.broadcast_to(
            np.asarray(bv, np.float32).astype(BF)[None, :], (P, EMB)))

        blobA = np.concatenate([
            xq_f,                                       # 98
            aux,                                        # 32
            id98.view(np.float32),                      # 49
            _f8cols(xq_f.astype(F8)),                   # 25
            _f8cols(_pack_w8i(wq, P)),                  # 128
            _f8cols(_pack_w8i(wk, P)),                  # 128
            _f8cols(xkv_f.astype(F8)),                  # 98
            _f8cols(xkvi),                              # 98
            _f8cols(wpcs),                              # 1
        ], axis=1)
        blobB = np.concatenate([
            _f8cols(_pack_w8(wv)),                      # 128
            bv_rep.view(np.float32),                    # 128
            _f8cols(_pack_w8i(wp, P)),                  # 128
        ], axis=1)
        if FFN_BF16:
            blobC = np.concatenate([
                _pack_wbf(w1).view(np.float32),
                _pack_wbf(w2).view(np.float32),
            ], axis=1)
        else:
            blobC = np.concatenate([
                _f8cols(_pack_w8i(w1, P)),                  # 512
                _f8cols(_pack_w8i(w2, P, mc_major=True)),   # 512
            ], axis=1)
        maps.append({
            'blobA': np.ascontiguousarray(blobA),
            'blobB': np.ascontiguousarray(blobB),
            'blobC': np.ascontiguousarray(blobC),
        })
    return maps


def _recon(x):
    x = x.reshape(14, 14, 16, 16)
    x = np.transpose(x, (2, 3, 0, 1))
    return x.reshape(1, 1, 224, 224)


def assemble(core_outs):
    ov = np.concatenate([core_outs[c].T for c in range(4)], axis=0)
    oi = np.concatenate([core_outs[c].T for c in range(4, 8)], axis=0)
    return np.concatenate([_recon(oi), _recon(ov)], axis=1).astype(np.float32)


def get_nc():
    if 'nc' not in _CACHE:
        _CACHE['nc'] = build_bass()
    return _CACHE['nc']


def kernel(**inputs):
    from concourse import bass_utils
    nc = get_nc()
    in_maps = make_in_maps(inputs)
    res = bass_utils.run_bass_kernel_spmd(nc, in_maps, core_ids=list(range(NCORES)))
    outs = [np.asarray(r['out'], np.float32) for r in res.results]
    return assemble(outs)
